# revision 33
# baseline (speedup 1.0000x reference)
"""Trainium2 kernel for nn_DeconvDecoder (moe_routing), 8-core data parallel.

Entire network runs on-device per core (256 samples/core):
  P1: trunk FC -> 6 switched-FC layers (routing folded into masked matmuls)
      -> d1 (deconv 4->8) -> d2a (1x1) -> d2b (depthwise deconv 8->16) -> DRAM
  P2: 4 switched-deconv layers at 16x16 (tap-packed K=128 matmuls, routing by
      maskz multiply), fp32 residual stream, batch-quartered
  P3: d3 (deconv 16->32), delta_y-packed K=96 matmuls
  P4: d4 (deconv 32->64), j-pair M-packing, writes final fp32 output

Activations/weights bf16 on the matmul path, fp32 PSUM + fp32 residuals.
"""

import sys

import numpy as np

for _p in ("/opt/trn_rl_repo", "/root/.axon_site/_ro/trn_rl_repo"):
    if _p not in sys.path:
        sys.path.append(_p)

import ml_dtypes

BF16 = ml_dtypes.bfloat16

B = 2048
NCORES = 8
BC = 256  # samples per core
NBR = 3
D = 1024
CH = 32
CHSM = 6

_NC_CACHE = {}


# --------------------------------------------------------------------------
# host-side weight packing (shared across cores)
# --------------------------------------------------------------------------


def _pack_weights(inp):
    f = np.float32
    w = {}

    # trunk: psum chunk k (8), M row u*64+c <-> feature c*16 + 2k+u
    wt = np.zeros((8, 16, 128), f)
    bt = np.zeros((64, 16), f)
    flw = np.asarray(inp["fc_latent_w"], f)
    flb = np.asarray(inp["fc_latent_b"], f)
    for k in range(8):
        for u in range(2):
            for c in range(64):
                feat = c * 16 + 2 * k + u
                wt[k, :10, u * 64 + c] = flw[:, feat]
                bt[c, k * 2 + u] = flb[feat]
    w["wtr"] = wt.transpose(1, 0, 2).copy().astype(BF16)  # [16,8,128]
    w["btr"] = bt

    # FC switches
    w1 = np.asarray(inp["fcsw_w1"], f)  # [6,3,1024,6]
    b1 = np.asarray(inp["fcsw_b1"], f)  # [6,3,6]
    w2 = np.asarray(inp["fcsw_w2"], f)  # [6,3,6,1024]
    b2 = np.asarray(inp["fcsw_b2"], f)  # [6,3,1024]
    W1a = np.zeros((6, 16, 64, 21), f)
    bfc1 = np.zeros((21, 6), f)
    W2e = np.zeros((6, 8, 21, 128), f)
    for j in range(6):
        for p in range(16):
            for c in range(64):
                feat = c * 16 + p
                for i in range(NBR):
                    W1a[j, p, c, i * 6 : i * 6 + 6] = w1[j, i, feat, :]
        bfc1[:18, j] = b1[j].reshape(18)
        bfc1[18:21, j] = 1.0
        for k in range(8):
            for u in range(2):
                for c in range(64):
                    feat = c * 16 + 2 * k + u
                    m = u * 64 + c
                    for i in range(NBR):
                        W2e[j, k, i * 6 : i * 6 + 6, m] = w2[j, i, :, feat]
                        W2e[j, k, 18 + i, m] = b2[j, i, feat]
    w["W1a"] = W1a.transpose(2, 0, 1, 3).copy().astype(BF16)  # [64,6,16,21]
    w["bfc1"] = bfc1
    w["W2e"] = W2e.transpose(2, 0, 1, 3).copy().astype(BF16)  # [21,6,8,128]

    # d1: [ry][tx][(a*64+cin),(rx*64+co)]
    wd1 = np.asarray(inp["w_d1"], f)  # [64,64,4,4]
    Wd1 = np.zeros((2, 3, 128, 128), f)
    for ry in range(2):
        for tx in range(3):
            for a in range(2):
                ky = 3 - 2 * a - ry
                for rx in range(2):
                    if not (0 <= tx - rx <= 1):
                        continue
                    kx = 3 + rx - 2 * tx
                    Wd1[ry, tx, a * 64 : a * 64 + 64, rx * 64 : rx * 64 + 64] = wd1[
                        :, :, ky, kx
                    ]
    w["Wd1"] = Wd1.transpose(2, 0, 1, 3).copy().astype(BF16)  # [128,2,3,128]
    w["bd1"] = np.asarray(inp["b_d1"], f).reshape(64, 1)

    w["Wd2a"] = np.asarray(inp["w_d2a"], f)[:, :, 0, 0].astype(BF16)  # [64,32]
    w["bd2a"] = np.asarray(inp["b_d2a"], f).reshape(32, 1)

    # d2b depthwise: [tx][(dl*32+c),((ry*2+rx)*32+co)], c==co
    wd2b = np.asarray(inp["w_d2b"], f)  # [32,1,4,4]
    Wd2b = np.zeros((3, 96, 128), f)
    for tx in range(3):
        for dl in range(3):
            for ry in range(2):
                if not (0 <= dl - ry <= 1):
                    continue
                ky = 3 + ry - 2 * dl
                for rx in range(2):
                    if not (0 <= tx - rx <= 1):
                        continue
                    kx = 3 + rx - 2 * tx
                    for c in range(32):
                        Wd2b[tx, dl * 32 + c, (ry * 2 + rx) * 32 + c] = wd2b[
                            c, 0, ky, kx
                        ]
    w["Wd2b"] = Wd2b.transpose(1, 0, 2).copy().astype(BF16)  # [96,3,128]
    w["bd2b"] = np.asarray(inp["b_d2b"], f).reshape(32, 1)

    # switched deconvs: tap d=(dy,dx), idx = dy*2+dx, weight tap (1-dy,1-dx)
    sw1 = np.asarray(inp["dcsw_w1"], f)  # [4,3,32,6,2,2]
    sb1 = np.asarray(inp["dcsw_b1"], f)  # [4,3,6]
    sw2 = np.asarray(inp["dcsw_w2"], f)  # [4,3,6,32,2,2]
    sb2 = np.asarray(inp["dcsw_b2"], f)  # [4,3,32]
    Wsw1 = np.zeros((4, 128, 32), f)  # M padded 18->32 (cols 18:32 zero)
    bsw1 = np.zeros((64, 4), f)  # per-partition bias for the g-merged psum
    Wsw2 = np.zeros((4, 75, 32), f)
    TAPORD1 = [(0, 1), (1, 1), (0, 0), (1, 0)]
    TAPORD2 = [(1, 1), (0, 1), (0, 0), (1, 0)]
    for s in range(4):
        for blk, (dy, dx) in enumerate(TAPORD1):
            for i in range(NBR):
                Wsw1[s, blk * 32 : blk * 32 + 32, i * 6 : i * 6 + 6] = sw1[
                    s, i, :, :, 1 - dy, 1 - dx
                ]
        for blk, (dy, dx) in enumerate(TAPORD2):
            for i in range(NBR):
                Wsw2[s, blk * 18 + i * 6 : blk * 18 + i * 6 + 6, :] = sw2[
                    s, i, :, :, 1 - dy, 1 - dx
                ]
        bsw1[0:18, s] = sb1[s].reshape(18)
        bsw1[32:50, s] = sb1[s].reshape(18)
        for i in range(NBR):
            Wsw2[s, 72 + i, :] = sb2[s, i, :]
    w["Wsw1"] = Wsw1.transpose(1, 0, 2).copy().astype(BF16)  # [128,4,32]
    w["bsw1"] = bsw1
    w["Wsw2"] = Wsw2.transpose(1, 0, 2).copy().astype(BF16)  # [75,4,32]

    # d3: [tx][(dl*32+cin),((ry*2+rx)*32+co)]
    wd3 = np.asarray(inp["w_d3"], f)  # [32,32,4,4]
    Wd3 = np.zeros((3, 96, 128), f)
    for tx in range(3):
        for dl in range(3):
            for ry in range(2):
                if not (0 <= dl - ry <= 1):
                    continue
                ky = 3 + ry - 2 * dl
                for rx in range(2):
                    if not (0 <= tx - rx <= 1):
                        continue
                    kx = 3 + rx - 2 * tx
                    Wd3[tx, dl * 32 : dl * 32 + 32, (ry * 2 + rx) * 32 : (ry * 2 + rx) * 32 + 32] = wd3[:, :, ky, kx]
    w["Wd3"] = Wd3.transpose(1, 0, 2).copy().astype(BF16)  # [96,3,128]
    bd3r = np.zeros((128, 1), f)
    bd3 = np.asarray(inp["b_d3"], f)
    for pr in range(4):
        bd3r[pr * 32 : pr * 32 + 32, 0] = bd3
    w["bd3r"] = bd3r

    # d4: [tx][(dl*32+c), ((u*2+ry)*2+rx)*3+co], dl = u + ty
    wd4 = np.asarray(inp["w_d4"], f)  # [32,3,4,4]
    Wd4 = np.zeros((3, 128, 24), f)
    for tx in range(3):
        for u in range(2):
            for ry in range(2):
                for ty in (ry, ry + 1):
                    dl = u + ty
                    ky = 3 + ry - 2 * ty
                    for rx in range(2):
                        if not (0 <= tx - rx <= 1):
                            continue
                        kx = 3 + rx - 2 * tx
                        m0 = ((u * 2 + ry) * 2 + rx) * 3
                        Wd4[tx, dl * 32 : dl * 32 + 32, m0 : m0 + 3] = wd4[:, :, ky, kx]
    w["Wd4"] = Wd4.transpose(1, 0, 2).copy().astype(BF16)  # [128,3,24]
    bd4r = np.zeros((24, 1), f)
    bd4 = np.asarray(inp["b_d4"], f)
    for g in range(8):
        bd4r[g * 3 : g * 3 + 3, 0] = bd4
    w["bd4r"] = bd4r
    return w


def _per_core_inputs(inp, core):
    f = np.float32
    sl = slice(core * BC, (core + 1) * BC)
    z2 = np.asarray(inp["z2"], f)[sl]  # [256,10]
    z2t = np.zeros((16, BC), f)
    z2t[:10] = z2.T
    ys = np.asarray(inp["ys_index"])[:, sl]  # [10,256]
    zs = np.asarray(inp["zs"], f)[:, sl, 0]  # [10,256]
    mz = np.zeros((21, 10, BC), f)
    for L in range(10):
        idx = 9 - L if L < 6 else 3 - (L - 6)
        for i in range(NBR):
            mzv = (ys[idx] == i).astype(f) * zs[idx]
            mz[18 + i, L] = mzv
            for hh in range(6):
                mz[i * 6 + hh, L] = mzv
    t = mz[18:21, 6:10, :].transpose(1, 0, 2)  # [4s, 3br, 256]
    t = t.reshape(4, 3, 8, 32).transpose(0, 2, 1, 3)  # [4s, 8chunk, 3br, 32]
    mzP = np.broadcast_to(
        t[:, :, :, None, None, :], (4, 8, 3, 16, 16, 32)
    ).astype(BF16)
    mzQ = np.zeros((4, 64, 4, 32), f)  # [qt, 32g+h, s, smp]
    for qt in range(4):
        for g in range(2):
            sl2 = slice(qt * 64 + g * 32, qt * 64 + g * 32 + 32)
            mzQ[qt, g * 32 : g * 32 + 18] = mz[0:18, 6:10, sl2]
    return {"z2t": z2t, "mzt": mz.astype(BF16),
            "mzP": np.ascontiguousarray(mzP), "mzQ": mzQ.astype(BF16)}


# --------------------------------------------------------------------------
# device program
# --------------------------------------------------------------------------


def _build_nc():
    import concourse.mybir as mybir
    from concourse import bacc
    from concourse.tile import TileContext

    f32 = mybir.dt.float32
    bf16 = mybir.dt.bfloat16
    AF = mybir.ActivationFunctionType
    ALU = mybir.AluOpType

    nc = bacc.Bacc("TRN2", target_bir_lowering=False, debug=False, num_devices=NCORES)

    # DRAM I/O
    z2t_d = nc.dram_tensor("z2t", [16, BC], f32, kind="ExternalInput").ap()
    mzt_d = nc.dram_tensor("mzt", [21, 10, BC], bf16, kind="ExternalInput").ap()
    mzP_d = nc.dram_tensor("mzP", [4, 8, 3, 16, 16, 32], bf16, kind="ExternalInput").ap()
    mzQ_d = nc.dram_tensor("mzQ", [4, 64, 4, 32], bf16, kind="ExternalInput").ap()
    wtr_d = nc.dram_tensor("wtr", [16, 8, 128], bf16, kind="ExternalInput").ap()
    btr_d = nc.dram_tensor("btr", [64, 16], f32, kind="ExternalInput").ap()
    W1a_d = nc.dram_tensor("W1a", [64, 6, 16, 21], bf16, kind="ExternalInput").ap()
    bfc1_d = nc.dram_tensor("bfc1", [21, 6], f32, kind="ExternalInput").ap()
    W2e_d = nc.dram_tensor("W2e", [21, 6, 8, 128], bf16, kind="ExternalInput").ap()
    Wd1_d = nc.dram_tensor("Wd1", [128, 2, 3, 128], bf16, kind="ExternalInput").ap()
    bd1_d = nc.dram_tensor("bd1", [64, 1], f32, kind="ExternalInput").ap()
    Wd2a_d = nc.dram_tensor("Wd2a", [64, 32], bf16, kind="ExternalInput").ap()
    bd2a_d = nc.dram_tensor("bd2a", [32, 1], f32, kind="ExternalInput").ap()
    Wd2b_d = nc.dram_tensor("Wd2b", [96, 3, 128], bf16, kind="ExternalInput").ap()
    bd2b_d = nc.dram_tensor("bd2b", [32, 1], f32, kind="ExternalInput").ap()
    Wsw1_d = nc.dram_tensor("Wsw1", [128, 4, 32], bf16, kind="ExternalInput").ap()
    bsw1_d = nc.dram_tensor("bsw1", [64, 4], f32, kind="ExternalInput").ap()
    Wsw2_d = nc.dram_tensor("Wsw2", [75, 4, 32], bf16, kind="ExternalInput").ap()
    Wd3_d = nc.dram_tensor("Wd3", [96, 3, 128], bf16, kind="ExternalInput").ap()
    bd3r_d = nc.dram_tensor("bd3r", [128, 1], f32, kind="ExternalInput").ap()
    Wd4_d = nc.dram_tensor("Wd4", [128, 3, 24], bf16, kind="ExternalInput").ap()
    bd4r_d = nc.dram_tensor("bd4r", [24, 1], f32, kind="ExternalInput").ap()

    # chunk-major internal layouts: per-partition runs are contiguous so DMA
    # descriptors are large (full-bandwidth) instead of 16-64B samples-inner
    mid1 = nc.dram_tensor("mid1", [8, 32, 16, 16, 32], bf16, kind="Internal").ap()
    mid2 = nc.dram_tensor("mid2", [8, 32, 16, 18, 32], bf16, kind="Internal").ap()
    mid3 = nc.dram_tensor("mid3", [8, 32, 2, 16, 34, 32], bf16, kind="Internal").ap()
    outD = nc.dram_tensor("out", [8, 24, 16, 32, 32], bf16, kind="ExternalOutput").ap()

    with TileContext(nc) as tc:
        with tc.tile_pool(name="wpool", bufs=1) as wp:
            # persistent weights (used by P2..P4)
            Wsw1 = wp.tile([128, 4, 32], bf16)
            nc.sync.dma_start(out=Wsw1[:], in_=Wsw1_d)
            bsw1 = wp.tile([64, 4], f32)
            nc.sync.dma_start(out=bsw1[:], in_=bsw1_d)
            Wsw2 = wp.tile([75, 4, 32], bf16)
            nc.sync.dma_start(out=Wsw2[:], in_=Wsw2_d)
            Wd3 = wp.tile([96, 3, 128], bf16)
            nc.sync.dma_start(out=Wd3[:], in_=Wd3_d)
            bd3r = wp.tile([128, 1], f32)
            nc.sync.dma_start(out=bd3r[:], in_=bd3r_d)
            Wd4 = wp.tile([128, 3, 24], bf16)
            nc.sync.dma_start(out=Wd4[:], in_=Wd4_d)
            bd4r = wp.tile([24, 1], f32)
            nc.sync.dma_start(out=bd4r[:], in_=bd4r_d)

            # ---------------- P1: trunk + FC switches ----------------
            with (
                tc.tile_pool(name="wp1", bufs=1) as wp1,
                tc.tile_pool(name="p1", bufs=1) as p1,
                tc.tile_pool(name="p1w", bufs=2) as p1w,
            ):
                # P1-only weights; pool closes after P1 to free SBUF for P2
                wtr = wp1.tile([16, 8, 128], bf16)
                nc.sync.dma_start(out=wtr[:], in_=wtr_d)
                btr = wp1.tile([64, 16], f32)
                nc.sync.dma_start(out=btr[:], in_=btr_d)
                W1a = wp1.tile([64, 6, 16, 21], bf16)
                nc.sync.dma_start(out=W1a[:], in_=W1a_d)
                bfc1 = wp1.tile([21, 6], f32)
                nc.sync.dma_start(out=bfc1[:], in_=bfc1_d)
                W2e = wp1.tile([21, 6, 8, 128], bf16)
                nc.sync.dma_start(out=W2e[:], in_=W2e_d)
                Wd1 = wp1.tile([128, 2, 3, 128], bf16)
                nc.sync.dma_start(out=Wd1[:], in_=Wd1_d)
                bd1 = wp1.tile([64, 1], f32)
                nc.sync.dma_start(out=bd1[:], in_=bd1_d)
                Wd2a = wp1.tile([64, 32], bf16)
                nc.sync.dma_start(out=Wd2a[:], in_=Wd2a_d)
                bd2a = wp1.tile([32, 1], f32)
                nc.sync.dma_start(out=bd2a[:], in_=bd2a_d)
                Wd2b = wp1.tile([96, 3, 128], bf16)
                nc.sync.dma_start(out=Wd2b[:], in_=Wd2b_d)
                bd2b = wp1.tile([32, 1], f32)
                nc.sync.dma_start(out=bd2b[:], in_=bd2b_d)
                mzt = wp1.tile([21, 10, BC], bf16)
                nc.sync.dma_start(out=mzt[:], in_=mzt_d)
                x_fc = p1.tile([64, BC, 4, 4], f32)
                fcpools = (
                    tc.tile_pool(name="ps_tr", bufs=2, space="PSUM"),
                    tc.tile_pool(name="ps_fc1", bufs=2, space="PSUM"),
                    tc.tile_pool(name="ps_fc2", bufs=4, space="PSUM"),
                )
                ps_tr = fcpools[0].__enter__()
                ps_fc1 = fcpools[1].__enter__()
                ps_fc2 = fcpools[2].__enter__()
                z2b = p1.tile([16, BC], bf16)
                z2f = p1w.tile([16, BC], f32)
                nc.sync.dma_start(out=z2f[:], in_=z2t_d)
                nc.vector.tensor_copy(z2b[:], z2f[:])
                for k in range(8):
                    ps = ps_tr.tile([128, BC], f32)
                    nc.tensor.matmul(ps[:], wtr[:, k, :], z2b[:], start=True, stop=True)
                    for u in range(2):
                        p = 2 * k + u
                        py, px = p // 4, p % 4
                        nc.scalar.activation(
                            x_fc[:, :, py, px],
                            ps[u * 64 : u * 64 + 64],
                            AF.Identity,
                            bias=btr[:, p : p + 1],
                        )

                for j in range(6):
                    h_fc = p1w.tile([64, BC, 4, 4], bf16, tag="hfc")
                    nc.scalar.activation(h_fc[:], x_fc[:], AF.Relu)
                    ps1 = ps_fc1.tile([21, BC], f32, tag="ps1")
                    for p in range(16):
                        py, px = p // 4, p % 4
                        nc.tensor.matmul(
                            ps1[:],
                            W1a[:, j, p, :],
                            h_fc[:, :, py, px],
                            start=(p == 0),
                            stop=(p == 15),
                        )
                    tt = p1w.tile([21, BC], bf16, tag="tt")
                    nc.scalar.activation(tt[:], ps1[:], AF.Relu, bias=bfc1[:, j : j + 1])
                    nc.vector.tensor_tensor(tt[:], tt[:], mzt[:, j, :], ALU.mult)
                    for k in range(8):
                        ps2 = ps_fc2.tile([128, BC], f32, tag="ps2")
                        nc.tensor.matmul(ps2[:], W2e[:, j, k, :], tt[:], start=True, stop=True)
                        for u in range(2):
                            p = 2 * k + u
                            py, px = p // 4, p % 4
                            nc.vector.tensor_tensor(
                                x_fc[:, :, py, px],
                                x_fc[:, :, py, px],
                                ps2[u * 64 : u * 64 + 64],
                                ALU.add,
                            )

                fcpools[2].__exit__(None, None, None)
                fcpools[1].__exit__(None, None, None)
                fcpools[0].__exit__(None, None, None)

                # ---------------- d1 ----------------
                with (
                    tc.tile_pool(name="pd1", bufs=1) as pd1,
                ):
                    h1d = pd1.tile([128, BC, 6, 6], bf16)
                    nc.vector.memset(h1d[:], 0.0)
                    nc.scalar.activation(h1d[0:64, :, 1:5, 1:5], x_fc[:], AF.Relu)
                    nc.scalar.activation(h1d[64:128, :, 0:4, 1:5], x_fc[:], AF.Relu)
                    h2 = pd1.tile([64, BC, 8, 8], bf16)
                    psp_d1 = tc.tile_pool(name="ps_d1", bufs=2, space="PSUM")
                    ps_d1 = psp_d1.__enter__()
                    for bc in range(8):
                        bs = slice(bc * 32, bc * 32 + 32)
                        for ry in range(2):
                            psd = ps_d1.tile([128, 32, 4, 4], f32, tag="psd1")
                            for tx in range(3):
                                nc.tensor.matmul(
                                    psd[:],
                                    Wd1[:, ry, tx, :],
                                    h1d[:, bs, ry : ry + 4, tx : tx + 4],
                                    start=(tx == 0),
                                    stop=(tx == 2),
                                )
                            for rx in range(2):
                                nc.scalar.activation(
                                    h2[:, bs, ry : 8 : 2, rx : 8 : 2],
                                    psd[rx * 64 : rx * 64 + 64],
                                    AF.Relu,
                                    bias=bd1[:],
                                )

                    psp_d1.__exit__(None, None, None)

                    # ---------------- d2a + d2b ----------------
                    h3d = pd1.tile([96, BC, 8, 10], bf16)
                    # zero only the halo regions (x cols 0/9 of the centre
                    # block, plus the y-rows the shifts leave unwritten)
                    nc.vector.memset(h3d[32:64, :, :, 0:10:9], 0.0)
                    nc.vector.memset(h3d[0:32, :, 0:1, :], 0.0)
                    nc.vector.memset(h3d[64:96, :, 7:8, :], 0.0)
                    psp_2a = tc.tile_pool(name="ps_d2a", bufs=2, space="PSUM")
                    ps_2a = psp_2a.__enter__()
                    for bc in range(8):
                        bs = slice(bc * 32, bc * 32 + 32)
                        psa = ps_2a.tile([32, 32, 8, 8], f32, tag="psd2a")
                        for sb in range(4):
                            ss = slice(bc * 32 + sb * 8, bc * 32 + sb * 8 + 8)
                            nc.tensor.matmul(
                                psa[:, sb * 8 : sb * 8 + 8],
                                Wd2a[:],
                                h2[:, ss],
                                start=True,
                                stop=True,
                            )
                        nc.scalar.activation(
                            h3d[32:64, bs, :, 1:9], psa[:], AF.Identity, bias=bd2a[:]
                        )
                    nc.sync.dma_start(
                        out=h3d[0:32, :, 1:8, :], in_=h3d[32:64, :, 0:7, :]
                    )
                    nc.sync.dma_start(
                        out=h3d[64:96, :, 0:7, :], in_=h3d[32:64, :, 1:8, :]
                    )
                    psp_2a.__exit__(None, None, None)
                    psp_2b = tc.tile_pool(name="ps_d2b", bufs=2, space="PSUM")
                    ps_2b = psp_2b.__enter__()
                    for bc in range(8):
                        bs = slice(bc * 32, bc * 32 + 32)
                        s1 = p1w.tile([32, 16, 16, 32], bf16, tag="s1")
                        psb = ps_2b.tile([128, 32, 8, 8], f32, tag="psd2b")
                        for sb in range(4):
                            ss = slice(bc * 32 + sb * 8, bc * 32 + sb * 8 + 8)
                            for tx in range(3):
                                nc.tensor.matmul(
                                    psb[:, sb * 8 : sb * 8 + 8],
                                    Wd2b[:, tx, :],
                                    h3d[:, ss, :, tx : tx + 8],
                                    start=(tx == 0),
                                    stop=(tx == 2),
                                )
                        for ry in range(2):
                            for rx in range(2):
                                dst = s1[:, ry : 16 : 2, rx : 16 : 2, :]
                                nc.scalar.activation(
                                    dst.transpose([0, 3, 1, 2]),
                                    psb[(ry * 2 + rx) * 32 : (ry * 2 + rx) * 32 + 32],
                                    AF.Identity,
                                    bias=bd2b[:],
                                )
                        nc.sync.dma_start(out=mid1[bc], in_=s1[:])
                    psp_2b.__exit__(None, None, None)

            # ---------------- P2: switched deconvs ----------------
            # 64-sample quarters; g-halves merged in PSUM partitions so the
            # evac / mask / residual-add ops run on 64 partitions per instr.
            with (
                tc.tile_pool(name="p2", bufs=2) as p2,
                tc.tile_pool(name="p2w", bufs=2) as p2w,
                tc.tile_pool(name="p2h", bufs=1) as p2h,
                tc.tile_pool(name="ps_sw1", bufs=4, space="PSUM") as ps_sw1,
                tc.tile_pool(name="ps_sw2", bufs=4, space="PSUM") as ps_sw2,
            ):
                for qt in range(4):  # 64-sample quarters
                    x4 = p2.tile([64, 16, 16, 32], bf16, tag="x4")
                    nc.sync.dma_start(out=x4[0:32], in_=mid1[2 * qt])
                    nc.sync.dma_start(out=x4[32:64], in_=mid1[2 * qt + 1])
                    mzqt = p2.tile([64, 4, 32], bf16, tag="mzqt")
                    nc.sync.dma_start(out=mzqt[:], in_=mzQ_d[qt])
                    for s in range(4):
                        hrelu = p2w.tile([64, 16, 16, 32], bf16, tag="hrelu")
                        tt = p2w.tile([64, 16, 16, 32], bf16, tag="tt")
                        h4s = [
                            p2h.tile([128, 16, 16, 32], bf16, tag=f"h4{g}",
                                     name=f"h4{g}")
                            for g in range(2)
                        ]
                        tt4s = [
                            p2h.tile([96, 16, 16, 32], bf16, tag=f"tt4{g}",
                                     name=f"tt4{g}")
                            for g in range(2)
                        ]
                        nc.scalar.activation(hrelu[:], x4[:], AF.Relu)
                        for g in range(2):
                            h4 = h4s[g]
                            g32 = slice(g * 32, g * 32 + 32)
                            # halo zeros (32-aligned partition bases)
                            nc.gpsimd.memset(h4[0:32, 0:1, :, :], 0.0)
                            nc.gpsimd.memset(h4[64:96, 0:1, :, :], 0.0)
                            nc.gpsimd.memset(h4[64:96, :, 0:1, :], 0.0)
                            nc.gpsimd.memset(h4[96:128, :, 0:1, :], 0.0)
                            # taps: 0:32=(0,1)y 32:64=(1,1) 64:96=(0,0)xy 96:128=(1,0)x
                            # Pool-issued (SWDGE) copies bypass the shared HWDGE
                            nc.gpsimd.dma_start(out=h4[32:64], in_=hrelu[g32])
                            nc.gpsimd.dma_start(
                                out=h4[0:32, 1:16, :, :], in_=hrelu[g32, 0:15, :, :]
                            )
                            nc.gpsimd.dma_start(
                                out=h4[64:96, 1:16, 1:16, :],
                                in_=hrelu[g32, 0:15, 0:15, :],
                            )
                            nc.gpsimd.dma_start(
                                out=h4[96:128, :, 1:16, :], in_=hrelu[g32, :, 0:15, :]
                            )
                        for q in range(8):  # 4-sample psum chunks
                            s4 = slice(q * 4, q * 4 + 4)
                            for yb in range(2):
                                yr = slice(yb * 8, yb * 8 + 8)
                                pss = ps_sw1.tile([64, 8, 16, 4], f32, tag="pss1")
                                for g in range(2):
                                    nc.tensor.matmul(
                                        pss[g * 32 : g * 32 + 32],
                                        Wsw1[:, s, :],
                                        h4s[g][:, yr, :, s4],
                                        start=True,
                                        stop=True,
                                    )
                                nc.scalar.activation(
                                    tt[:, yr, :, s4],
                                    pss[:],
                                    AF.Relu,
                                    bias=bsw1[:, s : s + 1],
                                )
                        mzb = (
                            mzqt[:, s, :]
                            .unsqueeze(1)
                            .unsqueeze(1)
                            .broadcast_to([64, 16, 16, 32])
                        )
                        nc.vector.tensor_tensor(tt[:], tt[:], mzb, ALU.mult)
                        for g in range(2):
                            tt4 = tt4s[g]
                            g18 = slice(g * 32, g * 32 + 18)
                            # rows: 0:18=(1,1) 18:36=(0,1)y 36:54=(0,0)xy 54:72=(1,0)x
                            nc.gpsimd.memset(tt4[0:64, 0:1, :, :], 0.0)
                            nc.gpsimd.memset(tt4[32:64, :, 0:1, :], 0.0)
                            nc.gpsimd.memset(tt4[64:96, :, 0:1, :], 0.0)
                            nc.sync.dma_start(out=tt4[72:75], in_=mzP_d[s, 2 * qt + g])
                            nc.sync.dma_start(out=tt4[0:18], in_=tt[g18])
                            nc.sync.dma_start(
                                out=tt4[18:36, 1:16, :, :], in_=tt[g18, 0:15, :, :]
                            )
                            nc.sync.dma_start(
                                out=tt4[36:54, 1:16, 1:16, :],
                                in_=tt[g18, 0:15, 0:15, :],
                            )
                            nc.sync.dma_start(
                                out=tt4[54:72, :, 1:16, :], in_=tt[g18, :, 0:15, :]
                            )
                        for q in range(8):
                            s4 = slice(q * 4, q * 4 + 4)
                            for yb in range(2):
                                yr = slice(yb * 8, yb * 8 + 8)
                                ps2 = ps_sw2.tile([64, 8, 16, 4], f32, tag="pss2")
                                for g in range(2):
                                    nc.tensor.matmul(
                                        ps2[g * 32 : g * 32 + 32],
                                        Wsw2[:, s, :],
                                        tt4s[g][0:75, yr, :, s4],
                                        start=True,
                                        stop=True,
                                    )
                                nc.vector.tensor_tensor(
                                    x4[:, yr, :, s4], x4[:, yr, :, s4], ps2[:], ALU.add
                                )
                    # store relu(x4) -> mid2 (x-halo baked)
                    hstp = p2w.tile([64, 16, 18, 32], bf16, tag="hst")
                    nc.gpsimd.memset(hstp[:, :, 0:1, :], 0.0)
                    nc.gpsimd.memset(hstp[:, :, 17:18, :], 0.0)
                    nc.scalar.activation(hstp[:, :, 1:17, :], x4[:], AF.Relu)
                    nc.sync.dma_start(out=mid2[2 * qt], in_=hstp[0:32])
                    nc.sync.dma_start(out=mid2[2 * qt + 1], in_=hstp[32:64])

            # ---------------- P3: d3 ----------------
            with (
                tc.tile_pool(name="p3", bufs=2) as p3,
                tc.tile_pool(name="ps_d3", bufs=2, space="PSUM") as ps_d3,
            ):
                for qt in range(8):  # 32-sample chunks
                    h5 = p3.tile([96, 16, 18, 32], bf16, tag="h5")
                    nc.sync.dma_start(out=h5[32:64], in_=mid2[qt])
                    nc.sync.dma_start(out=h5[0:32, 1:16, :, :], in_=mid2[qt, :, 0:15, :, :])
                    nc.sync.dma_start(out=h5[64:96, 0:15, :, :], in_=mid2[qt, :, 1:16, :, :])
                    nc.vector.memset(h5[0:32, 0:1, :, :], 0.0)
                    nc.vector.memset(h5[64:96, 15:16, :, :], 0.0)
                    s5a = p3.tile([32, 16, 34, 32], bf16, tag="s5a")
                    s5b = p3.tile([32, 16, 34, 32], bf16, tag="s5b")
                    for st in (s5a, s5b):
                        nc.vector.memset(st[:, :, 0:1, :], 0.0)
                        nc.vector.memset(st[:, :, 33:34, :], 0.0)
                    for q3 in range(4):  # 8-sample psum chunks
                        psd3 = ps_d3.tile([128, 16, 16, 8], f32, tag="psd3")
                        bs8 = slice(q3 * 8, q3 * 8 + 8)
                        for yb in range(4):
                            ys = slice(yb * 4, yb * 4 + 4)
                            for tx in range(3):
                                nc.tensor.matmul(
                                    psd3[:, ys, :, :],
                                    Wd3[:, tx, :],
                                    h5[:, ys, tx : tx + 16, bs8],
                                    start=(tx == 0),
                                    stop=(tx == 2),
                                )
                        nc.scalar.activation(
                            s5a[:, :, 1:33:2, bs8], psd3[0:32], AF.Relu, bias=bd3r[0:32]
                        )
                        nc.scalar.activation(
                            s5a[:, :, 2:34:2, bs8], psd3[32:64], AF.Relu, bias=bd3r[32:64]
                        )
                        nc.vector.tensor_scalar(
                            s5b[:, :, 1:33:2, bs8], psd3[64:96], bd3r[64:96], 0.0,
                            ALU.add, ALU.max,
                        )
                        nc.vector.tensor_scalar(
                            s5b[:, :, 2:34:2, bs8], psd3[96:128], bd3r[96:128], 0.0,
                            ALU.add, ALU.max,
                        )
                    nc.sync.dma_start(out=mid3[qt, :, 0], in_=s5a[:])
                    nc.sync.dma_start(out=mid3[qt, :, 1], in_=s5b[:])

            # ---------------- P4: d4 ----------------
            with (
                tc.tile_pool(name="p4", bufs=2) as p4,
                tc.tile_pool(name="ps_d4", bufs=2, space="PSUM") as ps_d4,
            ):
                for ck in range(8):  # 32-sample chunks
                    h6 = p4.tile([128, 16, 34, 32], bf16, tag="h6")
                    nc.sync.dma_start(out=h6[32:64], in_=mid3[ck, :, 0])
                    nc.sync.dma_start(
                        out=h6[0:32, 1:16, :, :], in_=mid3[ck, :, 1, 0:15, :, :]
                    )
                    nc.sync.dma_start(out=h6[64:96], in_=mid3[ck, :, 1])
                    nc.sync.dma_start(
                        out=h6[96:128, 0:15, :, :], in_=mid3[ck, :, 0, 1:16, :, :]
                    )
                    nc.vector.memset(h6[0:32, 0:1, :, :], 0.0)
                    nc.vector.memset(h6[96:128, 15:16, :, :], 0.0)
                    outSB = p4.tile([24, 16, 32, 32], bf16, tag="osb")
                    for q4 in range(8):
                        psd4 = ps_d4.tile([24, 16, 32, 4], f32, tag="psd4")
                        cs4 = slice(q4 * 4, q4 * 4 + 4)
                        for yb in range(4):
                            ys = slice(yb * 4, yb * 4 + 4)
                            for tx in range(3):
                                nc.tensor.matmul(
                                    psd4[:, ys, :, :],
                                    Wd4[:, tx, :],
                                    h6[:, ys, tx : tx + 32, cs4],
                                    start=(tx == 0),
                                    stop=(tx == 2),
                                )
                        nc.scalar.activation(
                            outSB[:, :, :, q4 * 4 : q4 * 4 + 4], psd4[:], AF.Identity,
                            bias=bd4r[:],
                        )
                    nc.sync.dma_start(out=outD[ck], in_=outSB[:])

    nc.compile()
    return nc


# --------------------------------------------------------------------------
# entry point
# --------------------------------------------------------------------------


def kernel(z2, ys_index, zs, fc_latent_w, fc_latent_b,
           fcsw_w1, fcsw_b1, fcsw_w2, fcsw_b2,
           dcsw_w1, dcsw_b1, dcsw_w2, dcsw_b2,
           w_d1, b_d1, w_d2a, b_d2a, w_d2b, b_d2b,
           w_d3, b_d3, w_d4, b_d4, _trace=False, _want_res=False):
    from concourse import bass_utils

    inp = dict(z2=z2, ys_index=ys_index, zs=zs, fc_latent_w=fc_latent_w,
               fc_latent_b=fc_latent_b, fcsw_w1=fcsw_w1, fcsw_b1=fcsw_b1,
               fcsw_w2=fcsw_w2, fcsw_b2=fcsw_b2, dcsw_w1=dcsw_w1,
               dcsw_b1=dcsw_b1, dcsw_w2=dcsw_w2, dcsw_b2=dcsw_b2,
               w_d1=w_d1, b_d1=b_d1, w_d2a=w_d2a, b_d2a=b_d2a, w_d2b=w_d2b,
               b_d2b=b_d2b, w_d3=w_d3, b_d3=b_d3, w_d4=w_d4, b_d4=b_d4)
    if "nc" not in _NC_CACHE:
        _NC_CACHE["nc"] = _build_nc()
    nc = _NC_CACHE["nc"]
    wshared = _pack_weights(inp)
    in_maps = []
    for c in range(NCORES):
        m = dict(wshared)
        m.update(_per_core_inputs(inp, c))
        in_maps.append(m)
    res = bass_utils.run_bass_kernel_spmd(nc, in_maps, list(range(NCORES)),
                                          trace=_trace)
    out = np.empty((B, 3, 64, 64), np.float32)
    # per-core outD: [8ck, 24, 16y, 32x, 32s]; partition p = g8*3 + oc,
    # g8 = (u*2+ry)*2+rx; y_out = 4*y + 2*u + ry; x_out = 2*x + rx
    ov = out.reshape(NCORES, 8, 32, 3, 16, 4, 32, 2)  # [core,ck,s,oc,y,yr,x,rx]
    for c in range(NCORES):
        a = np.asarray(res.results[c]["out"], np.float32).reshape(
            8, 8, 3, 16, 32, 32
        )  # [ck, g8, oc, y, x, s]
        a2 = a.transpose(0, 5, 2, 3, 1, 4).reshape(8, 32, 3, 16, 4, 2, 32)
        ov[c] = a2.transpose(0, 1, 2, 3, 4, 6, 5)
    if _want_res:
        return out, res
    return out



# revision 44
# speedup vs baseline: 1.1134x; 1.1134x over previous
"""Trainium2 kernel for nn_DeconvDecoder (moe_routing), 8-core data parallel.

Entire network runs on-device per core (256 samples/core):
  P1: trunk FC -> 6 switched-FC layers (routing folded into masked matmuls)
      -> d1 (deconv 4->8) -> d2a (1x1) -> d2b (depthwise deconv 8->16) -> DRAM
  P2: 4 switched-deconv layers at 16x16 (tap-packed K=128 matmuls, routing by
      maskz multiply), fp32 residual stream, batch-quartered
  P3: d3 (deconv 16->32), delta_y-packed K=96 matmuls
  P4: d4 (deconv 32->64), j-pair M-packing, writes final fp32 output

Activations/weights bf16 on the matmul path, fp32 PSUM + fp32 residuals.
"""

import sys

import numpy as np

for _p in ("/opt/trn_rl_repo", "/root/.axon_site/_ro/trn_rl_repo"):
    if _p not in sys.path:
        sys.path.append(_p)

import ml_dtypes

BF16 = ml_dtypes.bfloat16

B = 2048
NCORES = 8
BC = 256  # samples per core
NBR = 3
D = 1024
CH = 32
CHSM = 6

_NC_CACHE = {}


# --------------------------------------------------------------------------
# host-side weight packing (shared across cores)
# --------------------------------------------------------------------------


def _pack_weights(inp):
    f = np.float32
    w = {}

    # trunk: psum chunk k (8), M row u*64+c <-> feature c*16 + 2k+u
    wt = np.zeros((8, 16, 128), f)
    bt = np.zeros((64, 16), f)
    flw = np.asarray(inp["fc_latent_w"], f)
    flb = np.asarray(inp["fc_latent_b"], f)
    for k in range(8):
        for u in range(2):
            for c in range(64):
                feat = c * 16 + 2 * k + u
                wt[k, :10, u * 64 + c] = flw[:, feat]
                bt[c, k * 2 + u] = flb[feat]
    w["wtr"] = wt.transpose(1, 0, 2).copy().astype(BF16)  # [16,8,128]
    w["btr"] = bt

    # FC switches
    w1 = np.asarray(inp["fcsw_w1"], f)  # [6,3,1024,6]
    b1 = np.asarray(inp["fcsw_b1"], f)  # [6,3,6]
    w2 = np.asarray(inp["fcsw_w2"], f)  # [6,3,6,1024]
    b2 = np.asarray(inp["fcsw_b2"], f)  # [6,3,1024]
    W1a = np.zeros((6, 16, 64, 21), f)
    bfc1 = np.zeros((21, 6), f)
    W2e = np.zeros((6, 8, 21, 128), f)
    for j in range(6):
        for p in range(16):
            for c in range(64):
                feat = c * 16 + p
                for i in range(NBR):
                    W1a[j, p, c, i * 6 : i * 6 + 6] = w1[j, i, feat, :]
        bfc1[:18, j] = b1[j].reshape(18)
        bfc1[18:21, j] = 1.0
        for k in range(8):
            for u in range(2):
                for c in range(64):
                    feat = c * 16 + 2 * k + u
                    m = u * 64 + c
                    for i in range(NBR):
                        W2e[j, k, i * 6 : i * 6 + 6, m] = w2[j, i, :, feat]
                        W2e[j, k, 18 + i, m] = b2[j, i, feat]
    w["W1a"] = W1a.transpose(2, 0, 1, 3).copy().astype(BF16)  # [64,6,16,21]
    w["bfc1"] = bfc1
    w["W2e"] = W2e.transpose(2, 0, 1, 3).copy().astype(BF16)  # [21,6,8,128]

    # d1: [ry][tx][(a*64+cin),(rx*64+co)]
    wd1 = np.asarray(inp["w_d1"], f)  # [64,64,4,4]
    Wd1 = np.zeros((2, 3, 128, 128), f)
    for ry in range(2):
        for tx in range(3):
            for a in range(2):
                ky = 3 - 2 * a - ry
                for rx in range(2):
                    if not (0 <= tx - rx <= 1):
                        continue
                    kx = 3 + rx - 2 * tx
                    Wd1[ry, tx, a * 64 : a * 64 + 64, rx * 64 : rx * 64 + 64] = wd1[
                        :, :, ky, kx
                    ]
    w["Wd1"] = Wd1.transpose(2, 0, 1, 3).copy().astype(BF16)  # [128,2,3,128]
    w["bd1"] = np.asarray(inp["b_d1"], f).reshape(64, 1)

    w["Wd2a"] = np.asarray(inp["w_d2a"], f)[:, :, 0, 0].astype(BF16)  # [64,32]
    w["bd2a"] = np.asarray(inp["b_d2a"], f).reshape(32, 1)

    # d2b depthwise: [tx][(dl*32+c),((ry*2+rx)*32+co)], c==co
    wd2b = np.asarray(inp["w_d2b"], f)  # [32,1,4,4]
    Wd2b = np.zeros((3, 96, 128), f)
    for tx in range(3):
        for dl in range(3):
            for ry in range(2):
                if not (0 <= dl - ry <= 1):
                    continue
                ky = 3 + ry - 2 * dl
                for rx in range(2):
                    if not (0 <= tx - rx <= 1):
                        continue
                    kx = 3 + rx - 2 * tx
                    for c in range(32):
                        Wd2b[tx, dl * 32 + c, (ry * 2 + rx) * 32 + c] = wd2b[
                            c, 0, ky, kx
                        ]
    w["Wd2b"] = Wd2b.transpose(1, 0, 2).copy().astype(BF16)  # [96,3,128]
    w["bd2b"] = np.asarray(inp["b_d2b"], f).reshape(32, 1)

    # switched deconvs: tap d=(dy,dx), idx = dy*2+dx, weight tap (1-dy,1-dx)
    sw1 = np.asarray(inp["dcsw_w1"], f)  # [4,3,32,6,2,2]
    sb1 = np.asarray(inp["dcsw_b1"], f)  # [4,3,6]
    sw2 = np.asarray(inp["dcsw_w2"], f)  # [4,3,6,32,2,2]
    sb2 = np.asarray(inp["dcsw_b2"], f)  # [4,3,32]
    Wsw1 = np.zeros((4, 128, 32), f)  # M padded 18->32 (cols 18:32 zero)
    bsw1 = np.zeros((128, 4), f)  # per-partition bias, 4 merged 32-row blocks
    Wsw2 = np.zeros((4, 75, 32), f)
    TAPORD1 = [(0, 1), (1, 1), (0, 0), (1, 0)]
    TAPORD2 = [(1, 1), (0, 1), (0, 0), (1, 0)]
    for s in range(4):
        for blk, (dy, dx) in enumerate(TAPORD1):
            for i in range(NBR):
                Wsw1[s, blk * 32 : blk * 32 + 32, i * 6 : i * 6 + 6] = sw1[
                    s, i, :, :, 1 - dy, 1 - dx
                ]
        for blk, (dy, dx) in enumerate(TAPORD2):
            for i in range(NBR):
                Wsw2[s, blk * 18 + i * 6 : blk * 18 + i * 6 + 6, :] = sw2[
                    s, i, :, :, 1 - dy, 1 - dx
                ]
        for hg in range(4):
            bsw1[hg * 32 : hg * 32 + 18, s] = sb1[s].reshape(18)
        for i in range(NBR):
            Wsw2[s, 72 + i, :] = sb2[s, i, :]
    w["Wsw1"] = Wsw1.transpose(1, 0, 2).copy().astype(BF16)  # [128,4,32]
    w["bsw1"] = bsw1
    w["Wsw2"] = Wsw2.transpose(1, 0, 2).copy().astype(BF16)  # [75,4,32]

    # d3: [tx][(dl*32+cin),((ry*2+rx)*32+co)]
    wd3 = np.asarray(inp["w_d3"], f)  # [32,32,4,4]
    Wd3 = np.zeros((3, 96, 128), f)
    for tx in range(3):
        for dl in range(3):
            for ry in range(2):
                if not (0 <= dl - ry <= 1):
                    continue
                ky = 3 + ry - 2 * dl
                for rx in range(2):
                    if not (0 <= tx - rx <= 1):
                        continue
                    kx = 3 + rx - 2 * tx
                    Wd3[tx, dl * 32 : dl * 32 + 32, (ry * 2 + rx) * 32 : (ry * 2 + rx) * 32 + 32] = wd3[:, :, ky, kx]
    w["Wd3"] = Wd3.transpose(1, 0, 2).copy().astype(BF16)  # [96,3,128]
    bd3r = np.zeros((128, 1), f)
    bd3 = np.asarray(inp["b_d3"], f)
    for pr in range(4):
        bd3r[pr * 32 : pr * 32 + 32, 0] = bd3
    w["bd3r"] = bd3r

    # d4: [tx][(dl*32+c), ((u*2+ry)*2+rx)*3+co], dl = u + ty
    wd4 = np.asarray(inp["w_d4"], f)  # [32,3,4,4]
    Wd4 = np.zeros((3, 128, 24), f)
    for tx in range(3):
        for u in range(2):
            for ry in range(2):
                for ty in (ry, ry + 1):
                    dl = u + ty
                    ky = 3 + ry - 2 * ty
                    for rx in range(2):
                        if not (0 <= tx - rx <= 1):
                            continue
                        kx = 3 + rx - 2 * tx
                        m0 = ((u * 2 + ry) * 2 + rx) * 3
                        Wd4[tx, dl * 32 : dl * 32 + 32, m0 : m0 + 3] = wd4[:, :, ky, kx]
    w["Wd4"] = Wd4.transpose(1, 0, 2).copy().astype(BF16)  # [128,3,24]
    bd4r = np.zeros((24, 1), f)
    bd4 = np.asarray(inp["b_d4"], f)
    for g in range(8):
        bd4r[g * 3 : g * 3 + 3, 0] = bd4
    w["bd4r"] = bd4r
    return w


def _per_core_inputs(inp, core):
    f = np.float32
    sl = slice(core * BC, (core + 1) * BC)
    z2 = np.asarray(inp["z2"], f)[sl]  # [256,10]
    z2t = np.zeros((16, BC), f)
    z2t[:10] = z2.T
    ys = np.asarray(inp["ys_index"])[:, sl]  # [10,256]
    zs = np.asarray(inp["zs"], f)[:, sl, 0]  # [10,256]
    mz = np.zeros((21, 10, BC), f)
    for L in range(10):
        idx = 9 - L if L < 6 else 3 - (L - 6)
        for i in range(NBR):
            mzv = (ys[idx] == i).astype(f) * zs[idx]
            mz[18 + i, L] = mzv
            for hh in range(6):
                mz[i * 6 + hh, L] = mzv
    t = mz[18:21, 6:10, :].transpose(1, 0, 2)  # [4s, 3br, 256]
    t = t.reshape(4, 3, 8, 32).transpose(0, 2, 1, 3)  # [4s, 8chunk, 3br, 32]
    mzP = np.broadcast_to(
        t[:, :, :, None, None, :], (4, 8, 3, 16, 16, 32)
    ).astype(BF16)
    mzQ = np.zeros((2, 128, 4, 32), f)  # [qp, 32hg+j, s, smp]
    for qp in range(2):
        for hg in range(4):
            sl2 = slice(qp * 128 + hg * 32, qp * 128 + hg * 32 + 32)
            mzQ[qp, hg * 32 : hg * 32 + 18] = mz[0:18, 6:10, sl2]
    return {"z2t": z2t, "mzt": mz.astype(BF16),
            "mzP": np.ascontiguousarray(mzP), "mzQ": mzQ.astype(BF16)}


# --------------------------------------------------------------------------
# device program
# --------------------------------------------------------------------------


def _build_nc():
    import concourse.mybir as mybir
    from concourse import bacc
    from concourse.tile import TileContext

    f32 = mybir.dt.float32
    bf16 = mybir.dt.bfloat16
    AF = mybir.ActivationFunctionType
    ALU = mybir.AluOpType

    nc = bacc.Bacc("TRN2", target_bir_lowering=False, debug=False, num_devices=NCORES)

    # DRAM I/O
    z2t_d = nc.dram_tensor("z2t", [16, BC], f32, kind="ExternalInput").ap()
    mzt_d = nc.dram_tensor("mzt", [21, 10, BC], bf16, kind="ExternalInput").ap()
    mzP_d = nc.dram_tensor("mzP", [4, 8, 3, 16, 16, 32], bf16, kind="ExternalInput").ap()
    mzQ_d = nc.dram_tensor("mzQ", [2, 128, 4, 32], bf16, kind="ExternalInput").ap()
    wtr_d = nc.dram_tensor("wtr", [16, 8, 128], bf16, kind="ExternalInput").ap()
    btr_d = nc.dram_tensor("btr", [64, 16], f32, kind="ExternalInput").ap()
    W1a_d = nc.dram_tensor("W1a", [64, 6, 16, 21], bf16, kind="ExternalInput").ap()
    bfc1_d = nc.dram_tensor("bfc1", [21, 6], f32, kind="ExternalInput").ap()
    W2e_d = nc.dram_tensor("W2e", [21, 6, 8, 128], bf16, kind="ExternalInput").ap()
    Wd1_d = nc.dram_tensor("Wd1", [128, 2, 3, 128], bf16, kind="ExternalInput").ap()
    bd1_d = nc.dram_tensor("bd1", [64, 1], f32, kind="ExternalInput").ap()
    Wd2a_d = nc.dram_tensor("Wd2a", [64, 32], bf16, kind="ExternalInput").ap()
    bd2a_d = nc.dram_tensor("bd2a", [32, 1], f32, kind="ExternalInput").ap()
    Wd2b_d = nc.dram_tensor("Wd2b", [96, 3, 128], bf16, kind="ExternalInput").ap()
    bd2b_d = nc.dram_tensor("bd2b", [32, 1], f32, kind="ExternalInput").ap()
    Wsw1_d = nc.dram_tensor("Wsw1", [128, 4, 32], bf16, kind="ExternalInput").ap()
    bsw1_d = nc.dram_tensor("bsw1", [128, 4], f32, kind="ExternalInput").ap()
    Wsw2_d = nc.dram_tensor("Wsw2", [75, 4, 32], bf16, kind="ExternalInput").ap()
    Wd3_d = nc.dram_tensor("Wd3", [96, 3, 128], bf16, kind="ExternalInput").ap()
    bd3r_d = nc.dram_tensor("bd3r", [128, 1], f32, kind="ExternalInput").ap()
    Wd4_d = nc.dram_tensor("Wd4", [128, 3, 24], bf16, kind="ExternalInput").ap()
    bd4r_d = nc.dram_tensor("bd4r", [24, 1], f32, kind="ExternalInput").ap()

    # chunk-major internal layouts: per-partition runs are contiguous so DMA
    # descriptors are large (full-bandwidth) instead of 16-64B samples-inner
    mid1 = nc.dram_tensor("mid1", [8, 32, 16, 16, 32], bf16, kind="Internal").ap()
    mid2 = nc.dram_tensor("mid2", [8, 32, 16, 18, 32], bf16, kind="Internal").ap()
    mid3 = nc.dram_tensor("mid3", [8, 32, 2, 16, 34, 32], bf16, kind="Internal").ap()
    outD = nc.dram_tensor("out", [8, 24, 16, 32, 32], bf16, kind="ExternalOutput").ap()

    with TileContext(nc) as tc:
        with tc.tile_pool(name="wpool", bufs=1) as wp:
            # persistent weights (used by P2..P4)
            Wsw1 = wp.tile([128, 4, 32], bf16)
            nc.sync.dma_start(out=Wsw1[:], in_=Wsw1_d)
            bsw1 = wp.tile([128, 4], f32)
            nc.sync.dma_start(out=bsw1[:], in_=bsw1_d)
            Wsw2 = wp.tile([75, 4, 32], bf16)
            nc.sync.dma_start(out=Wsw2[:], in_=Wsw2_d)
            Wd3 = wp.tile([96, 3, 128], bf16)
            nc.sync.dma_start(out=Wd3[:], in_=Wd3_d)
            bd3r = wp.tile([128, 1], f32)
            nc.sync.dma_start(out=bd3r[:], in_=bd3r_d)
            Wd4 = wp.tile([128, 3, 24], bf16)
            nc.sync.dma_start(out=Wd4[:], in_=Wd4_d)
            bd4r = wp.tile([24, 1], f32)
            nc.sync.dma_start(out=bd4r[:], in_=bd4r_d)

            # ---------------- P1: trunk + FC switches ----------------
            with (
                tc.tile_pool(name="wp1", bufs=1) as wp1,
                tc.tile_pool(name="p1", bufs=1) as p1,
                tc.tile_pool(name="p1w", bufs=2) as p1w,
            ):
                # P1-only weights; pool closes after P1 to free SBUF for P2
                wtr = wp1.tile([16, 8, 128], bf16)
                nc.sync.dma_start(out=wtr[:], in_=wtr_d)
                btr = wp1.tile([64, 16], f32)
                nc.sync.dma_start(out=btr[:], in_=btr_d)
                W1a = wp1.tile([64, 6, 16, 21], bf16)
                nc.sync.dma_start(out=W1a[:], in_=W1a_d)
                bfc1 = wp1.tile([21, 6], f32)
                nc.sync.dma_start(out=bfc1[:], in_=bfc1_d)
                W2e = wp1.tile([21, 6, 8, 128], bf16)
                nc.sync.dma_start(out=W2e[:], in_=W2e_d)
                Wd1 = wp1.tile([128, 2, 3, 128], bf16)
                nc.sync.dma_start(out=Wd1[:], in_=Wd1_d)
                bd1 = wp1.tile([64, 1], f32)
                nc.sync.dma_start(out=bd1[:], in_=bd1_d)
                Wd2a = wp1.tile([64, 32], bf16)
                nc.sync.dma_start(out=Wd2a[:], in_=Wd2a_d)
                bd2a = wp1.tile([32, 1], f32)
                nc.sync.dma_start(out=bd2a[:], in_=bd2a_d)
                Wd2b = wp1.tile([96, 3, 128], bf16)
                nc.sync.dma_start(out=Wd2b[:], in_=Wd2b_d)
                bd2b = wp1.tile([32, 1], f32)
                nc.sync.dma_start(out=bd2b[:], in_=bd2b_d)
                mzt = wp1.tile([21, 10, BC], bf16)
                nc.sync.dma_start(out=mzt[:], in_=mzt_d)
                x_fc = p1.tile([64, BC, 4, 4], f32)
                fcpools = (
                    tc.tile_pool(name="ps_tr", bufs=2, space="PSUM"),
                    tc.tile_pool(name="ps_fc1", bufs=2, space="PSUM"),
                    tc.tile_pool(name="ps_fc2", bufs=4, space="PSUM"),
                )
                ps_tr = fcpools[0].__enter__()
                ps_fc1 = fcpools[1].__enter__()
                ps_fc2 = fcpools[2].__enter__()
                z2b = p1.tile([16, BC], bf16)
                z2f = p1w.tile([16, BC], f32)
                nc.sync.dma_start(out=z2f[:], in_=z2t_d)
                nc.vector.tensor_copy(z2b[:], z2f[:])
                for k in range(8):
                    ps = ps_tr.tile([128, BC], f32)
                    nc.tensor.matmul(ps[:], wtr[:, k, :], z2b[:], start=True, stop=True)
                    for u in range(2):
                        p = 2 * k + u
                        py, px = p // 4, p % 4
                        nc.scalar.activation(
                            x_fc[:, :, py, px],
                            ps[u * 64 : u * 64 + 64],
                            AF.Identity,
                            bias=btr[:, p : p + 1],
                        )

                for j in range(6):
                    h_fc = p1w.tile([64, BC, 4, 4], bf16, tag="hfc")
                    nc.scalar.activation(h_fc[:], x_fc[:], AF.Relu)
                    ps1 = ps_fc1.tile([21, BC], f32, tag="ps1")
                    for p in range(16):
                        py, px = p // 4, p % 4
                        nc.tensor.matmul(
                            ps1[:],
                            W1a[:, j, p, :],
                            h_fc[:, :, py, px],
                            start=(p == 0),
                            stop=(p == 15),
                        )
                    tt = p1w.tile([21, BC], bf16, tag="tt")
                    nc.scalar.activation(tt[:], ps1[:], AF.Relu, bias=bfc1[:, j : j + 1])
                    nc.vector.tensor_tensor(tt[:], tt[:], mzt[:, j, :], ALU.mult)
                    for k in range(8):
                        ps2 = ps_fc2.tile([128, BC], f32, tag="ps2")
                        nc.tensor.matmul(ps2[:], W2e[:, j, k, :], tt[:], start=True, stop=True)
                        for u in range(2):
                            p = 2 * k + u
                            py, px = p // 4, p % 4
                            nc.vector.tensor_tensor(
                                x_fc[:, :, py, px],
                                x_fc[:, :, py, px],
                                ps2[u * 64 : u * 64 + 64],
                                ALU.add,
                            )

                fcpools[2].__exit__(None, None, None)
                fcpools[1].__exit__(None, None, None)
                fcpools[0].__exit__(None, None, None)

                # ---------------- d1 ----------------
                with (
                    tc.tile_pool(name="pd1", bufs=1) as pd1,
                ):
                    h1d = pd1.tile([128, BC, 6, 6], bf16)
                    nc.vector.memset(h1d[:], 0.0)
                    nc.scalar.activation(h1d[0:64, :, 1:5, 1:5], x_fc[:], AF.Relu)
                    nc.scalar.activation(h1d[64:128, :, 0:4, 1:5], x_fc[:], AF.Relu)
                    h2 = pd1.tile([64, BC, 8, 8], bf16)
                    psp_d1 = tc.tile_pool(name="ps_d1", bufs=2, space="PSUM")
                    ps_d1 = psp_d1.__enter__()
                    for bc in range(8):
                        bs = slice(bc * 32, bc * 32 + 32)
                        for ry in range(2):
                            psd = ps_d1.tile([128, 32, 4, 4], f32, tag="psd1")
                            for tx in range(3):
                                nc.tensor.matmul(
                                    psd[:],
                                    Wd1[:, ry, tx, :],
                                    h1d[:, bs, ry : ry + 4, tx : tx + 4],
                                    start=(tx == 0),
                                    stop=(tx == 2),
                                )
                            for rx in range(2):
                                nc.scalar.activation(
                                    h2[:, bs, ry : 8 : 2, rx : 8 : 2],
                                    psd[rx * 64 : rx * 64 + 64],
                                    AF.Relu,
                                    bias=bd1[:],
                                )

                    psp_d1.__exit__(None, None, None)

                    # ---------------- d2a + d2b ----------------
                    h3d = pd1.tile([96, BC, 8, 10], bf16)
                    # zero only the halo regions (x cols 0/9 of the centre
                    # block, plus the y-rows the shifts leave unwritten)
                    nc.vector.memset(h3d[32:64, :, :, 0:10:9], 0.0)
                    nc.vector.memset(h3d[0:32, :, 0:1, :], 0.0)
                    nc.vector.memset(h3d[64:96, :, 7:8, :], 0.0)
                    psp_2a = tc.tile_pool(name="ps_d2a", bufs=2, space="PSUM")
                    ps_2a = psp_2a.__enter__()
                    for bc in range(8):
                        bs = slice(bc * 32, bc * 32 + 32)
                        psa = ps_2a.tile([32, 32, 8, 8], f32, tag="psd2a")
                        for sb in range(4):
                            ss = slice(bc * 32 + sb * 8, bc * 32 + sb * 8 + 8)
                            nc.tensor.matmul(
                                psa[:, sb * 8 : sb * 8 + 8],
                                Wd2a[:],
                                h2[:, ss],
                                start=True,
                                stop=True,
                            )
                        nc.scalar.activation(
                            h3d[32:64, bs, :, 1:9], psa[:], AF.Identity, bias=bd2a[:]
                        )
                    nc.sync.dma_start(
                        out=h3d[0:32, :, 1:8, :], in_=h3d[32:64, :, 0:7, :]
                    )
                    nc.sync.dma_start(
                        out=h3d[64:96, :, 0:7, :], in_=h3d[32:64, :, 1:8, :]
                    )
                    psp_2a.__exit__(None, None, None)
                    psp_2b = tc.tile_pool(name="ps_d2b", bufs=2, space="PSUM")
                    ps_2b = psp_2b.__enter__()
                    for bc in range(8):
                        bs = slice(bc * 32, bc * 32 + 32)
                        s1 = p1w.tile([32, 16, 16, 32], bf16, tag="s1")
                        psb = ps_2b.tile([128, 32, 8, 8], f32, tag="psd2b")
                        for sb in range(4):
                            ss = slice(bc * 32 + sb * 8, bc * 32 + sb * 8 + 8)
                            for tx in range(3):
                                nc.tensor.matmul(
                                    psb[:, sb * 8 : sb * 8 + 8],
                                    Wd2b[:, tx, :],
                                    h3d[:, ss, :, tx : tx + 8],
                                    start=(tx == 0),
                                    stop=(tx == 2),
                                )
                        for ry in range(2):
                            for rx in range(2):
                                dst = s1[:, ry : 16 : 2, rx : 16 : 2, :]
                                nc.scalar.activation(
                                    dst.transpose([0, 3, 1, 2]),
                                    psb[(ry * 2 + rx) * 32 : (ry * 2 + rx) * 32 + 32],
                                    AF.Identity,
                                    bias=bd2b[:],
                                )
                        nc.sync.dma_start(out=mid1[bc], in_=s1[:])
                    psp_2b.__exit__(None, None, None)

            # ---------------- P2: switched deconvs ----------------
            # 64-sample quarters; g-halves merged in PSUM partitions so the
            # evac / mask / residual-add ops run on 64 partitions per instr.
            # Two quarters pair-packed per 128-partition tile: partition row
            # p = h*64 + g*32 + c (h = quarter-in-pair, g = 32-sample group).
            # Every elementwise op covers 128 samples; conv psums use all 4
            # PE column positions (0/32/64/96).
            with (
                tc.tile_pool(name="p2", bufs=1) as p2,
                tc.tile_pool(name="p2w", bufs=2) as p2w,
                tc.tile_pool(name="p2s", bufs=1) as p2s,
                tc.tile_pool(name="p2h", bufs=1) as p2h,
                tc.tile_pool(name="ps_sw1", bufs=4, space="PSUM") as ps_sw1,
                tc.tile_pool(name="ps_sw2", bufs=4, space="PSUM") as ps_sw2,
            ):
                for qp in range(2):  # pairs of 64-sample quarters
                    x4 = p2.tile([128, 16, 16, 32], bf16, tag="x4")
                    for hg in range(4):
                        nc.sync.dma_start(
                            out=x4[hg * 32 : hg * 32 + 32], in_=mid1[4 * qp + hg]
                        )
                    mzqt = p2.tile([128, 4, 32], bf16, tag="mzqt")
                    nc.sync.dma_start(out=mzqt[:], in_=mzQ_d[qp])
                    for s in range(4):
                        # hrelu and tt have disjoint lifetimes; share one
                        # 2-buffer tag so they alternate the same memory
                        hrelu = p2w.tile([128, 16, 16, 32], bf16, tag="sc")
                        tt = p2w.tile([128, 16, 16, 32], bf16, tag="sc")
                        h4s = [
                            p2h.tile([128, 16, 16, 32], bf16, tag=f"h4{hg}",
                                     name=f"h4{hg}")
                            for hg in range(4)
                        ]
                        tt4s = [
                            p2h.tile([96, 16, 16, 32], bf16, tag=f"tt4{hg}",
                                     name=f"tt4{hg}")
                            for hg in range(4)
                        ]
                        nc.scalar.activation(hrelu[:], x4[:], AF.Relu)
                        for hg in range(4):
                            h4 = h4s[hg]
                            g32 = slice(hg * 32, hg * 32 + 32)
                            eng = nc.gpsimd if hg % 2 == 0 else nc.sync
                            # halo zeros (32-aligned partition bases)
                            nc.gpsimd.memset(h4[0:32, 0:1, :, :], 0.0)
                            nc.gpsimd.memset(h4[64:96, 0:1, :, :], 0.0)
                            nc.gpsimd.memset(h4[64:96, :, 0:1, :], 0.0)
                            nc.gpsimd.memset(h4[96:128, :, 0:1, :], 0.0)
                            # taps: 0:32=(0,1)y 32:64=(1,1) 64:96=(0,0)xy 96:128=(1,0)x
                            eng.dma_start(out=h4[32:64], in_=hrelu[g32])
                            eng.dma_start(
                                out=h4[0:32, 1:16, :, :], in_=hrelu[g32, 0:15, :, :]
                            )
                            eng.dma_start(
                                out=h4[64:96, 1:16, 1:16, :],
                                in_=hrelu[g32, 0:15, 0:15, :],
                            )
                            eng.dma_start(
                                out=h4[96:128, :, 1:16, :], in_=hrelu[g32, :, 0:15, :]
                            )
                        for q in range(8):  # 4-sample psum chunks
                            s4 = slice(q * 4, q * 4 + 4)
                            for yb in range(2):
                                yr = slice(yb * 8, yb * 8 + 8)
                                pss = ps_sw1.tile([128, 8, 16, 4], f32, tag="pss1")
                                for hg in range(4):
                                    nc.tensor.matmul(
                                        pss[hg * 32 : hg * 32 + 32],
                                        Wsw1[:, s, :],
                                        h4s[hg][:, yr, :, s4],
                                        start=True,
                                        stop=True,
                                        tile_position=(0, hg * 32),
                                    )
                                nc.scalar.activation(
                                    tt[:, yr, :, s4],
                                    pss[:],
                                    AF.Relu,
                                    bias=bsw1[:, s : s + 1],
                                )
                        mzb = (
                            mzqt[:, s, :]
                            .unsqueeze(1)
                            .unsqueeze(1)
                            .broadcast_to([128, 16, 16, 32])
                        )
                        nc.vector.tensor_tensor(tt[:], tt[:], mzb, ALU.mult)
                        for hg in range(4):
                            tt4 = tt4s[hg]
                            g18 = slice(hg * 32, hg * 32 + 18)
                            eng = nc.gpsimd if hg % 2 == 1 else nc.sync
                            # rows: 0:18=(1,1) 18:36=(0,1)y 36:54=(0,0)xy 54:72=(1,0)x
                            nc.gpsimd.memset(tt4[0:64, 0:1, :, :], 0.0)
                            nc.gpsimd.memset(tt4[32:64, :, 0:1, :], 0.0)
                            nc.gpsimd.memset(tt4[64:96, :, 0:1, :], 0.0)
                            nc.sync.dma_start(
                                out=tt4[72:75], in_=mzP_d[s, 4 * qp + hg]
                            )
                            eng.dma_start(out=tt4[0:18], in_=tt[g18])
                            eng.dma_start(
                                out=tt4[18:36, 1:16, :, :], in_=tt[g18, 0:15, :, :]
                            )
                            eng.dma_start(
                                out=tt4[36:54, 1:16, 1:16, :],
                                in_=tt[g18, 0:15, 0:15, :],
                            )
                            eng.dma_start(
                                out=tt4[54:72, :, 1:16, :], in_=tt[g18, :, 0:15, :]
                            )
                        for q in range(8):
                            s4 = slice(q * 4, q * 4 + 4)
                            for yb in range(2):
                                yr = slice(yb * 8, yb * 8 + 8)
                                ps2 = ps_sw2.tile([128, 8, 16, 4], f32, tag="pss2")
                                for hg in range(4):
                                    nc.tensor.matmul(
                                        ps2[hg * 32 : hg * 32 + 32],
                                        Wsw2[:, s, :],
                                        tt4s[hg][0:75, yr, :, s4],
                                        start=True,
                                        stop=True,
                                        tile_position=(0, hg * 32),
                                    )
                                nc.vector.tensor_tensor(
                                    x4[:, yr, :, s4], x4[:, yr, :, s4], ps2[:], ALU.add
                                )
                    # store relu(x4) -> mid2 (x-halo baked)
                    hstp = p2s.tile([128, 16, 18, 32], bf16, tag="hst")
                    nc.gpsimd.memset(hstp[:, :, 0:1, :], 0.0)
                    nc.gpsimd.memset(hstp[:, :, 17:18, :], 0.0)
                    nc.scalar.activation(hstp[:, :, 1:17, :], x4[:], AF.Relu)
                    for hg in range(4):
                        nc.sync.dma_start(
                            out=mid2[4 * qp + hg], in_=hstp[hg * 32 : hg * 32 + 32]
                        )

            # ---------------- P3: d3 ----------------
            with (
                tc.tile_pool(name="p3", bufs=2) as p3,
                tc.tile_pool(name="ps_d3", bufs=2, space="PSUM") as ps_d3,
            ):
                for qt in range(8):  # 32-sample chunks
                    h5 = p3.tile([96, 16, 18, 32], bf16, tag="h5")
                    nc.sync.dma_start(out=h5[32:64], in_=mid2[qt])
                    nc.sync.dma_start(out=h5[0:32, 1:16, :, :], in_=mid2[qt, :, 0:15, :, :])
                    nc.sync.dma_start(out=h5[64:96, 0:15, :, :], in_=mid2[qt, :, 1:16, :, :])
                    nc.vector.memset(h5[0:32, 0:1, :, :], 0.0)
                    nc.vector.memset(h5[64:96, 15:16, :, :], 0.0)
                    s5a = p3.tile([32, 16, 34, 32], bf16, tag="s5a")
                    s5b = p3.tile([32, 16, 34, 32], bf16, tag="s5b")
                    for st in (s5a, s5b):
                        nc.vector.memset(st[:, :, 0:1, :], 0.0)
                        nc.vector.memset(st[:, :, 33:34, :], 0.0)
                    for q3 in range(4):  # 8-sample psum chunks
                        psd3 = ps_d3.tile([128, 16, 16, 8], f32, tag="psd3")
                        bs8 = slice(q3 * 8, q3 * 8 + 8)
                        for yb in range(4):
                            ys = slice(yb * 4, yb * 4 + 4)
                            for tx in range(3):
                                nc.tensor.matmul(
                                    psd3[:, ys, :, :],
                                    Wd3[:, tx, :],
                                    h5[:, ys, tx : tx + 16, bs8],
                                    start=(tx == 0),
                                    stop=(tx == 2),
                                )
                        nc.scalar.activation(
                            s5a[:, :, 1:33:2, bs8], psd3[0:32], AF.Relu, bias=bd3r[0:32]
                        )
                        nc.scalar.activation(
                            s5a[:, :, 2:34:2, bs8], psd3[32:64], AF.Relu, bias=bd3r[32:64]
                        )
                        nc.vector.tensor_scalar(
                            s5b[:, :, 1:33:2, bs8], psd3[64:96], bd3r[64:96], 0.0,
                            ALU.add, ALU.max,
                        )
                        nc.vector.tensor_scalar(
                            s5b[:, :, 2:34:2, bs8], psd3[96:128], bd3r[96:128], 0.0,
                            ALU.add, ALU.max,
                        )
                    nc.sync.dma_start(out=mid3[qt, :, 0], in_=s5a[:])
                    nc.sync.dma_start(out=mid3[qt, :, 1], in_=s5b[:])

            # ---------------- P4: d4 ----------------
            with (
                tc.tile_pool(name="p4", bufs=2) as p4,
                tc.tile_pool(name="ps_d4", bufs=2, space="PSUM") as ps_d4,
            ):
                for ck in range(8):  # 32-sample chunks
                    h6 = p4.tile([128, 16, 34, 32], bf16, tag="h6")
                    nc.sync.dma_start(out=h6[32:64], in_=mid3[ck, :, 0])
                    nc.sync.dma_start(
                        out=h6[0:32, 1:16, :, :], in_=mid3[ck, :, 1, 0:15, :, :]
                    )
                    nc.sync.dma_start(out=h6[64:96], in_=mid3[ck, :, 1])
                    nc.sync.dma_start(
                        out=h6[96:128, 0:15, :, :], in_=mid3[ck, :, 0, 1:16, :, :]
                    )
                    nc.vector.memset(h6[0:32, 0:1, :, :], 0.0)
                    nc.vector.memset(h6[96:128, 15:16, :, :], 0.0)
                    outSB = p4.tile([24, 16, 32, 32], bf16, tag="osb")
                    for q4 in range(8):
                        psd4 = ps_d4.tile([24, 16, 32, 4], f32, tag="psd4")
                        cs4 = slice(q4 * 4, q4 * 4 + 4)
                        for yb in range(4):
                            ys = slice(yb * 4, yb * 4 + 4)
                            for tx in range(3):
                                nc.tensor.matmul(
                                    psd4[:, ys, :, :],
                                    Wd4[:, tx, :],
                                    h6[:, ys, tx : tx + 32, cs4],
                                    start=(tx == 0),
                                    stop=(tx == 2),
                                )
                        nc.scalar.activation(
                            outSB[:, :, :, q4 * 4 : q4 * 4 + 4], psd4[:], AF.Identity,
                            bias=bd4r[:],
                        )
                    nc.sync.dma_start(out=outD[ck], in_=outSB[:])

    nc.compile()
    return nc


# --------------------------------------------------------------------------
# entry point
# --------------------------------------------------------------------------


def kernel(z2, ys_index, zs, fc_latent_w, fc_latent_b,
           fcsw_w1, fcsw_b1, fcsw_w2, fcsw_b2,
           dcsw_w1, dcsw_b1, dcsw_w2, dcsw_b2,
           w_d1, b_d1, w_d2a, b_d2a, w_d2b, b_d2b,
           w_d3, b_d3, w_d4, b_d4, _trace=False, _want_res=False):
    from concourse import bass_utils

    inp = dict(z2=z2, ys_index=ys_index, zs=zs, fc_latent_w=fc_latent_w,
               fc_latent_b=fc_latent_b, fcsw_w1=fcsw_w1, fcsw_b1=fcsw_b1,
               fcsw_w2=fcsw_w2, fcsw_b2=fcsw_b2, dcsw_w1=dcsw_w1,
               dcsw_b1=dcsw_b1, dcsw_w2=dcsw_w2, dcsw_b2=dcsw_b2,
               w_d1=w_d1, b_d1=b_d1, w_d2a=w_d2a, b_d2a=b_d2a, w_d2b=w_d2b,
               b_d2b=b_d2b, w_d3=w_d3, b_d3=b_d3, w_d4=w_d4, b_d4=b_d4)
    if "nc" not in _NC_CACHE:
        _NC_CACHE["nc"] = _build_nc()
    nc = _NC_CACHE["nc"]
    wshared = _pack_weights(inp)
    in_maps = []
    for c in range(NCORES):
        m = dict(wshared)
        m.update(_per_core_inputs(inp, c))
        in_maps.append(m)
    res = bass_utils.run_bass_kernel_spmd(nc, in_maps, list(range(NCORES)),
                                          trace=_trace)
    out = np.empty((B, 3, 64, 64), np.float32)
    # per-core outD: [8ck, 24, 16y, 32x, 32s]; partition p = g8*3 + oc,
    # g8 = (u*2+ry)*2+rx; y_out = 4*y + 2*u + ry; x_out = 2*x + rx
    ov = out.reshape(NCORES, 8, 32, 3, 16, 4, 32, 2)  # [core,ck,s,oc,y,yr,x,rx]
    for c in range(NCORES):
        a = np.asarray(res.results[c]["out"], np.float32).reshape(
            8, 8, 3, 16, 32, 32
        )  # [ck, g8, oc, y, x, s]
        a2 = a.transpose(0, 5, 2, 3, 1, 4).reshape(8, 32, 3, 16, 4, 2, 32)
        ov[c] = a2.transpose(0, 1, 2, 3, 4, 6, 5)
    if _want_res:
        return out, res
    return out



# revision 52
# speedup vs baseline: 1.3091x; 1.1758x over previous
"""Trainium2 kernel for nn_DeconvDecoder (moe_routing), 8-core data parallel.

Entire network runs on-device per core (256 samples/core):
  P1: trunk FC -> 6 switched-FC layers (routing folded into masked matmuls)
      -> d1 (deconv 4->8) -> d2a (1x1) -> d2b (depthwise deconv 8->16) -> DRAM
  P2: 4 switched-deconv layers at 16x16 (tap-packed K=128 matmuls, routing by
      maskz multiply), fp32 residual stream, batch-quartered
  P3: d3 (deconv 16->32), delta_y-packed K=96 matmuls
  P4: d4 (deconv 32->64), j-pair M-packing, writes final fp32 output

Activations/weights bf16 on the matmul path, fp32 PSUM + fp32 residuals.
"""

import sys

import numpy as np

for _p in ("/opt/trn_rl_repo", "/root/.axon_site/_ro/trn_rl_repo"):
    if _p not in sys.path:
        sys.path.append(_p)

import ml_dtypes

BF16 = ml_dtypes.bfloat16

B = 2048
NCORES = 8
BC = 256  # samples per core
NBR = 3
D = 1024
CH = 32
CHSM = 6

_NC_CACHE = {}


# --------------------------------------------------------------------------
# host-side weight packing (shared across cores)
# --------------------------------------------------------------------------


def _pack_weights(inp):
    f = np.float32
    w = {}

    # trunk: psum chunk k (8), M row u*64+c <-> feature c*16 + 2k+u
    wt = np.zeros((8, 16, 128), f)
    bt = np.zeros((64, 16), f)
    flw = np.asarray(inp["fc_latent_w"], f)
    flb = np.asarray(inp["fc_latent_b"], f)
    for k in range(8):
        for u in range(2):
            for c in range(64):
                feat = c * 16 + 2 * k + u
                wt[k, :10, u * 64 + c] = flw[:, feat]
                bt[c, k * 2 + u] = flb[feat]
    w["wtr"] = wt.transpose(1, 0, 2).copy().astype(BF16)  # [16,8,128]
    w["btr"] = bt

    # FC switches
    w1 = np.asarray(inp["fcsw_w1"], f)  # [6,3,1024,6]
    b1 = np.asarray(inp["fcsw_b1"], f)  # [6,3,6]
    w2 = np.asarray(inp["fcsw_w2"], f)  # [6,3,6,1024]
    b2 = np.asarray(inp["fcsw_b2"], f)  # [6,3,1024]
    W1a = np.zeros((6, 16, 64, 21), f)
    bfc1 = np.zeros((21, 6), f)
    W2e = np.zeros((6, 8, 21, 128), f)
    for j in range(6):
        for p in range(16):
            for c in range(64):
                feat = c * 16 + p
                for i in range(NBR):
                    W1a[j, p, c, i * 6 : i * 6 + 6] = w1[j, i, feat, :]
        bfc1[:18, j] = b1[j].reshape(18)
        bfc1[18:21, j] = 1.0
        for k in range(8):
            for u in range(2):
                for c in range(64):
                    feat = c * 16 + 2 * k + u
                    m = u * 64 + c
                    for i in range(NBR):
                        W2e[j, k, i * 6 : i * 6 + 6, m] = w2[j, i, :, feat]
                        W2e[j, k, 18 + i, m] = b2[j, i, feat]
    w["W1a"] = W1a.transpose(2, 0, 1, 3).copy().astype(BF16)  # [64,6,16,21]
    w["bfc1"] = bfc1
    w["W2e"] = W2e.transpose(2, 0, 1, 3).copy().astype(BF16)  # [21,6,8,128]

    # d1: [ry][tx][(a*64+cin),(rx*64+co)]
    wd1 = np.asarray(inp["w_d1"], f)  # [64,64,4,4]
    Wd1 = np.zeros((2, 3, 128, 128), f)
    for ry in range(2):
        for tx in range(3):
            for a in range(2):
                ky = 3 - 2 * a - ry
                for rx in range(2):
                    if not (0 <= tx - rx <= 1):
                        continue
                    kx = 3 + rx - 2 * tx
                    Wd1[ry, tx, a * 64 : a * 64 + 64, rx * 64 : rx * 64 + 64] = wd1[
                        :, :, ky, kx
                    ]
    w["Wd1"] = Wd1.transpose(2, 0, 1, 3).copy().astype(BF16)  # [128,2,3,128]
    w["bd1"] = np.asarray(inp["b_d1"], f).reshape(64, 1)

    w["Wd2a"] = np.asarray(inp["w_d2a"], f)[:, :, 0, 0].astype(BF16)  # [64,32]
    w["bd2a"] = np.asarray(inp["b_d2a"], f).reshape(32, 1)

    # d2b depthwise: [tx][(dl*32+c),((ry*2+rx)*32+co)], c==co
    wd2b = np.asarray(inp["w_d2b"], f)  # [32,1,4,4]
    Wd2b = np.zeros((3, 96, 128), f)
    for tx in range(3):
        for dl in range(3):
            for ry in range(2):
                if not (0 <= dl - ry <= 1):
                    continue
                ky = 3 + ry - 2 * dl
                for rx in range(2):
                    if not (0 <= tx - rx <= 1):
                        continue
                    kx = 3 + rx - 2 * tx
                    for c in range(32):
                        Wd2b[tx, dl * 32 + c, (ry * 2 + rx) * 32 + c] = wd2b[
                            c, 0, ky, kx
                        ]
    w["Wd2b"] = Wd2b.transpose(1, 0, 2).copy().astype(BF16)  # [96,3,128]
    w["bd2b"] = np.asarray(inp["b_d2b"], f).reshape(32, 1)

    # switched deconvs: tap d=(dy,dx), idx = dy*2+dx, weight tap (1-dy,1-dx)
    sw1 = np.asarray(inp["dcsw_w1"], f)  # [4,3,32,6,2,2]
    sb1 = np.asarray(inp["dcsw_b1"], f)  # [4,3,6]
    sw2 = np.asarray(inp["dcsw_w2"], f)  # [4,3,6,32,2,2]
    sb2 = np.asarray(inp["dcsw_b2"], f)  # [4,3,32]
    Wsw1 = np.zeros((4, 128, 32), f)  # M padded 18->32 (cols 18:32 zero)
    bsw1 = np.zeros((128, 4), f)  # per-partition bias, 4 merged 32-row blocks
    Wsw2 = np.zeros((4, 75, 32), f)
    TAPORD1 = [(0, 1), (1, 1), (0, 0), (1, 0)]
    TAPORD2 = [(1, 1), (0, 1), (0, 0), (1, 0)]
    for s in range(4):
        for blk, (dy, dx) in enumerate(TAPORD1):
            for i in range(NBR):
                Wsw1[s, blk * 32 : blk * 32 + 32, i * 6 : i * 6 + 6] = sw1[
                    s, i, :, :, 1 - dy, 1 - dx
                ]
        for blk, (dy, dx) in enumerate(TAPORD2):
            for i in range(NBR):
                Wsw2[s, blk * 18 + i * 6 : blk * 18 + i * 6 + 6, :] = sw2[
                    s, i, :, :, 1 - dy, 1 - dx
                ]
        for hg in range(4):
            bsw1[hg * 32 : hg * 32 + 18, s] = sb1[s].reshape(18)
        for i in range(NBR):
            Wsw2[s, 72 + i, :] = sb2[s, i, :]
    w["Wsw1"] = Wsw1.transpose(1, 0, 2).copy().astype(BF16)  # [128,4,32]
    w["bsw1"] = bsw1
    w["Wsw2"] = Wsw2.transpose(1, 0, 2).copy().astype(BF16)  # [75,4,32]

    # d3: [tx][(dl*32+cin),(rx*64+ry*32+co)]  (rx-major M so the P3 evac
    # can address each rx half-block with a single 64-partition AP)
    wd3 = np.asarray(inp["w_d3"], f)  # [32,32,4,4]
    Wd3 = np.zeros((3, 96, 128), f)
    for tx in range(3):
        for dl in range(3):
            for ry in range(2):
                if not (0 <= dl - ry <= 1):
                    continue
                ky = 3 + ry - 2 * dl
                for rx in range(2):
                    if not (0 <= tx - rx <= 1):
                        continue
                    kx = 3 + rx - 2 * tx
                    m0 = rx * 64 + ry * 32
                    Wd3[tx, dl * 32 : dl * 32 + 32, m0 : m0 + 32] = wd3[:, :, ky, kx]
    w["Wd3"] = Wd3.transpose(1, 0, 2).copy().astype(BF16)  # [96,3,128]
    bd3r = np.zeros((128, 1), f)
    bd3 = np.asarray(inp["b_d3"], f)
    for pr in range(4):
        bd3r[pr * 32 : pr * 32 + 32, 0] = bd3
    w["bd3r"] = bd3r

    # d4 x-banded: out x-blocks XB = 4*xd + j' packed into M = 96 = j'*24+m;
    # K packs 4 in-band input cols (xm) x 32c.  Per dl-round r (y-tap
    # replica), chunk A covers cols xm 0..3 of xd=xbb, chunk B cols 0..1 of
    # xd=xbb+1.  Base tap table Wd4t[tx][dl*32+c, m] as before.
    wd4 = np.asarray(inp["w_d4"], f)  # [32,3,4,4]
    Wd4t = np.zeros((3, 128, 24), f)
    for tx in range(3):
        for u in range(2):
            for ry in range(2):
                for ty in (ry, ry + 1):
                    dl = u + ty
                    ky = 3 + ry - 2 * ty
                    for rx in range(2):
                        if not (0 <= tx - rx <= 1):
                            continue
                        kx = 3 + rx - 2 * tx
                        m0 = ((u * 2 + ry) * 2 + rx) * 3
                        Wd4t[tx, dl * 32 : dl * 32 + 32, m0 : m0 + 3] = wd4[:, :, ky, kx]
    Wd4A = np.zeros((4, 128, 96), f)
    Wd4B = np.zeros((4, 64, 96), f)
    for r in range(4):  # r = dl
        for jp in range(4):
            for xm in range(4):
                tx = xm - jp
                if 0 <= tx <= 2:
                    Wd4A[r, xm * 32 : xm * 32 + 32, jp * 24 : jp * 24 + 24] = Wd4t[
                        tx, r * 32 : r * 32 + 32, :
                    ]
            for b2 in range(2):
                tx = 4 + b2 - jp
                if 0 <= tx <= 2:
                    Wd4B[r, b2 * 32 : b2 * 32 + 32, jp * 24 : jp * 24 + 24] = Wd4t[
                        tx, r * 32 : r * 32 + 32, :
                    ]
    w["Wd4A"] = Wd4A.transpose(1, 0, 2).copy().astype(BF16)  # [128,4,96]
    w["Wd4B"] = Wd4B.transpose(1, 0, 2).copy().astype(BF16)  # [64,4,96]
    bd4r = np.zeros((96, 1), f)
    bd4 = np.asarray(inp["b_d4"], f)
    for g in range(32):
        bd4r[g * 3 : g * 3 + 3, 0] = bd4
    w["bd4r"] = bd4r
    return w


def _per_core_inputs(inp, core):
    f = np.float32
    sl = slice(core * BC, (core + 1) * BC)
    z2 = np.asarray(inp["z2"], f)[sl]  # [256,10]
    z2t = np.zeros((16, BC), f)
    z2t[:10] = z2.T
    ys = np.asarray(inp["ys_index"])[:, sl]  # [10,256]
    zs = np.asarray(inp["zs"], f)[:, sl, 0]  # [10,256]
    mz = np.zeros((21, 10, BC), f)
    for L in range(10):
        idx = 9 - L if L < 6 else 3 - (L - 6)
        for i in range(NBR):
            mzv = (ys[idx] == i).astype(f) * zs[idx]
            mz[18 + i, L] = mzv
            for hh in range(6):
                mz[i * 6 + hh, L] = mzv
    t = mz[18:21, 6:10, :].transpose(1, 0, 2)  # [4s, 3br, 256]
    t = t.reshape(4, 3, 8, 32).transpose(0, 2, 1, 3)  # [4s, 8chunk, 3br, 32]
    mzP = np.broadcast_to(
        t[:, :, :, None, None, :], (4, 8, 3, 16, 16, 32)
    ).astype(BF16)
    mzQ = np.zeros((2, 128, 4, 32), f)  # [qp, 32hg+j, s, smp]
    for qp in range(2):
        for hg in range(4):
            sl2 = slice(qp * 128 + hg * 32, qp * 128 + hg * 32 + 32)
            mzQ[qp, hg * 32 : hg * 32 + 18] = mz[0:18, 6:10, sl2]
    return {"z2t": z2t, "mzt": mz.astype(BF16),
            "mzP": np.ascontiguousarray(mzP), "mzQ": mzQ.astype(BF16)}


# --------------------------------------------------------------------------
# device program
# --------------------------------------------------------------------------


def _build_nc():
    import concourse.mybir as mybir
    from concourse import bacc
    from concourse.tile import TileContext

    f32 = mybir.dt.float32
    bf16 = mybir.dt.bfloat16
    AF = mybir.ActivationFunctionType
    ALU = mybir.AluOpType

    nc = bacc.Bacc("TRN2", target_bir_lowering=False, debug=False, num_devices=NCORES)

    # DRAM I/O
    z2t_d = nc.dram_tensor("z2t", [16, BC], f32, kind="ExternalInput").ap()
    mzt_d = nc.dram_tensor("mzt", [21, 10, BC], bf16, kind="ExternalInput").ap()
    mzP_d = nc.dram_tensor("mzP", [4, 8, 3, 16, 16, 32], bf16, kind="ExternalInput").ap()
    mzQ_d = nc.dram_tensor("mzQ", [2, 128, 4, 32], bf16, kind="ExternalInput").ap()
    wtr_d = nc.dram_tensor("wtr", [16, 8, 128], bf16, kind="ExternalInput").ap()
    btr_d = nc.dram_tensor("btr", [64, 16], f32, kind="ExternalInput").ap()
    W1a_d = nc.dram_tensor("W1a", [64, 6, 16, 21], bf16, kind="ExternalInput").ap()
    bfc1_d = nc.dram_tensor("bfc1", [21, 6], f32, kind="ExternalInput").ap()
    W2e_d = nc.dram_tensor("W2e", [21, 6, 8, 128], bf16, kind="ExternalInput").ap()
    Wd1_d = nc.dram_tensor("Wd1", [128, 2, 3, 128], bf16, kind="ExternalInput").ap()
    bd1_d = nc.dram_tensor("bd1", [64, 1], f32, kind="ExternalInput").ap()
    Wd2a_d = nc.dram_tensor("Wd2a", [64, 32], bf16, kind="ExternalInput").ap()
    bd2a_d = nc.dram_tensor("bd2a", [32, 1], f32, kind="ExternalInput").ap()
    Wd2b_d = nc.dram_tensor("Wd2b", [96, 3, 128], bf16, kind="ExternalInput").ap()
    bd2b_d = nc.dram_tensor("bd2b", [32, 1], f32, kind="ExternalInput").ap()
    Wsw1_d = nc.dram_tensor("Wsw1", [128, 4, 32], bf16, kind="ExternalInput").ap()
    bsw1_d = nc.dram_tensor("bsw1", [128, 4], f32, kind="ExternalInput").ap()
    Wsw2_d = nc.dram_tensor("Wsw2", [75, 4, 32], bf16, kind="ExternalInput").ap()
    Wd3_d = nc.dram_tensor("Wd3", [96, 3, 128], bf16, kind="ExternalInput").ap()
    bd3r_d = nc.dram_tensor("bd3r", [128, 1], f32, kind="ExternalInput").ap()
    Wd4A_d = nc.dram_tensor("Wd4A", [128, 4, 96], bf16, kind="ExternalInput").ap()
    Wd4B_d = nc.dram_tensor("Wd4B", [64, 4, 96], bf16, kind="ExternalInput").ap()
    bd4r_d = nc.dram_tensor("bd4r", [96, 1], f32, kind="ExternalInput").ap()

    # chunk-major internal layouts: per-partition runs are contiguous so DMA
    # descriptors are large (full-bandwidth) instead of 16-64B samples-inner
    mid1 = nc.dram_tensor("mid1", [8, 32, 16, 16, 32], bf16, kind="Internal").ap()
    mid2 = nc.dram_tensor("mid2", [8, 32, 16, 18, 32], bf16, kind="Internal").ap()
    # mid3: [ck, rx*64+ry*32+c, y_halo 18, xmHalf 2, xd 9, s 32]
    mid3 = nc.dram_tensor("mid3", [8, 128, 18, 2, 9, 32], bf16, kind="Internal").ap()
    outD = nc.dram_tensor("out", [8, 96, 16, 8, 32], bf16, kind="ExternalOutput").ap()

    with TileContext(nc) as tc:
        with tc.tile_pool(name="wpool", bufs=1) as wp:
            # persistent weights (used by P2..P4)
            Wsw1 = wp.tile([128, 4, 32], bf16)
            nc.sync.dma_start(out=Wsw1[:], in_=Wsw1_d)
            bsw1 = wp.tile([128, 4], f32)
            nc.sync.dma_start(out=bsw1[:], in_=bsw1_d)
            Wsw2 = wp.tile([75, 4, 32], bf16)
            nc.sync.dma_start(out=Wsw2[:], in_=Wsw2_d)
            Wd3 = wp.tile([96, 3, 128], bf16)
            nc.sync.dma_start(out=Wd3[:], in_=Wd3_d)
            bd3r = wp.tile([128, 1], f32)
            nc.sync.dma_start(out=bd3r[:], in_=bd3r_d)
            Wd4A = wp.tile([128, 4, 96], bf16)
            nc.sync.dma_start(out=Wd4A[:], in_=Wd4A_d)
            Wd4B = wp.tile([64, 4, 96], bf16)
            nc.sync.dma_start(out=Wd4B[:], in_=Wd4B_d)
            bd4r = wp.tile([96, 1], f32)
            nc.sync.dma_start(out=bd4r[:], in_=bd4r_d)

            # ---------------- P1: trunk + FC switches ----------------
            with (
                tc.tile_pool(name="wp1", bufs=1) as wp1,
                tc.tile_pool(name="p1", bufs=1) as p1,
                tc.tile_pool(name="p1w", bufs=2) as p1w,
            ):
                # P1-only weights; pool closes after P1 to free SBUF for P2
                wtr = wp1.tile([16, 8, 128], bf16)
                nc.sync.dma_start(out=wtr[:], in_=wtr_d)
                btr = wp1.tile([64, 16], f32)
                nc.sync.dma_start(out=btr[:], in_=btr_d)
                W1a = wp1.tile([64, 6, 16, 21], bf16)
                nc.sync.dma_start(out=W1a[:], in_=W1a_d)
                bfc1 = wp1.tile([21, 6], f32)
                nc.sync.dma_start(out=bfc1[:], in_=bfc1_d)
                W2e = wp1.tile([21, 6, 8, 128], bf16)
                nc.sync.dma_start(out=W2e[:], in_=W2e_d)
                Wd1 = wp1.tile([128, 2, 3, 128], bf16)
                nc.sync.dma_start(out=Wd1[:], in_=Wd1_d)
                bd1 = wp1.tile([64, 1], f32)
                nc.sync.dma_start(out=bd1[:], in_=bd1_d)
                Wd2a = wp1.tile([64, 32], bf16)
                nc.sync.dma_start(out=Wd2a[:], in_=Wd2a_d)
                bd2a = wp1.tile([32, 1], f32)
                nc.sync.dma_start(out=bd2a[:], in_=bd2a_d)
                Wd2b = wp1.tile([96, 3, 128], bf16)
                nc.sync.dma_start(out=Wd2b[:], in_=Wd2b_d)
                bd2b = wp1.tile([32, 1], f32)
                nc.sync.dma_start(out=bd2b[:], in_=bd2b_d)
                mzt = wp1.tile([21, 10, BC], bf16)
                nc.sync.dma_start(out=mzt[:], in_=mzt_d)
                x_fc = p1.tile([64, BC, 4, 4], f32)
                fcpools = (
                    tc.tile_pool(name="ps_tr", bufs=2, space="PSUM"),
                    tc.tile_pool(name="ps_fc1", bufs=2, space="PSUM"),
                    tc.tile_pool(name="ps_fc2", bufs=4, space="PSUM"),
                )
                ps_tr = fcpools[0].__enter__()
                ps_fc1 = fcpools[1].__enter__()
                ps_fc2 = fcpools[2].__enter__()
                z2b = p1.tile([16, BC], bf16)
                z2f = p1w.tile([16, BC], f32)
                nc.sync.dma_start(out=z2f[:], in_=z2t_d)
                nc.vector.tensor_copy(z2b[:], z2f[:])
                for k in range(8):
                    ps = ps_tr.tile([128, BC], f32)
                    nc.tensor.matmul(ps[:], wtr[:, k, :], z2b[:], start=True, stop=True)
                    for u in range(2):
                        p = 2 * k + u
                        py, px = p // 4, p % 4
                        nc.scalar.activation(
                            x_fc[:, :, py, px],
                            ps[u * 64 : u * 64 + 64],
                            AF.Identity,
                            bias=btr[:, p : p + 1],
                        )

                for j in range(6):
                    h_fc = p1w.tile([64, BC, 4, 4], bf16, tag="hfc")
                    nc.scalar.activation(h_fc[:], x_fc[:], AF.Relu)
                    ps1 = ps_fc1.tile([21, BC], f32, tag="ps1")
                    for p in range(16):
                        py, px = p // 4, p % 4
                        nc.tensor.matmul(
                            ps1[:],
                            W1a[:, j, p, :],
                            h_fc[:, :, py, px],
                            start=(p == 0),
                            stop=(p == 15),
                        )
                    tt = p1w.tile([21, BC], bf16, tag="tt")
                    nc.scalar.activation(tt[:], ps1[:], AF.Relu, bias=bfc1[:, j : j + 1])
                    nc.vector.tensor_tensor(tt[:], tt[:], mzt[:, j, :], ALU.mult)
                    for k in range(8):
                        ps2 = ps_fc2.tile([128, BC], f32, tag="ps2")
                        nc.tensor.matmul(ps2[:], W2e[:, j, k, :], tt[:], start=True, stop=True)
                        for u in range(2):
                            p = 2 * k + u
                            py, px = p // 4, p % 4
                            nc.vector.tensor_tensor(
                                x_fc[:, :, py, px],
                                x_fc[:, :, py, px],
                                ps2[u * 64 : u * 64 + 64],
                                ALU.add,
                            )

                fcpools[2].__exit__(None, None, None)
                fcpools[1].__exit__(None, None, None)
                fcpools[0].__exit__(None, None, None)

                # ---------------- d1 ----------------
                with (
                    tc.tile_pool(name="pd1", bufs=1) as pd1,
                ):
                    h1d = pd1.tile([128, BC, 6, 6], bf16)
                    nc.vector.memset(h1d[:], 0.0)
                    nc.scalar.activation(h1d[0:64, :, 1:5, 1:5], x_fc[:], AF.Relu)
                    nc.scalar.activation(h1d[64:128, :, 0:4, 1:5], x_fc[:], AF.Relu)
                    h2 = pd1.tile([64, BC, 8, 8], bf16)
                    psp_d1 = tc.tile_pool(name="ps_d1", bufs=2, space="PSUM")
                    ps_d1 = psp_d1.__enter__()
                    for bc in range(8):
                        bs = slice(bc * 32, bc * 32 + 32)
                        for ry in range(2):
                            psd = ps_d1.tile([128, 32, 4, 4], f32, tag="psd1")
                            for tx in range(3):
                                nc.tensor.matmul(
                                    psd[:],
                                    Wd1[:, ry, tx, :],
                                    h1d[:, bs, ry : ry + 4, tx : tx + 4],
                                    start=(tx == 0),
                                    stop=(tx == 2),
                                )
                            for rx in range(2):
                                nc.scalar.activation(
                                    h2[:, bs, ry : 8 : 2, rx : 8 : 2],
                                    psd[rx * 64 : rx * 64 + 64],
                                    AF.Relu,
                                    bias=bd1[:],
                                )

                    psp_d1.__exit__(None, None, None)

                    # ---------------- d2a + d2b ----------------
                    h3d = pd1.tile([96, BC, 8, 10], bf16)
                    # zero only the halo regions (x cols 0/9 of the centre
                    # block, plus the y-rows the shifts leave unwritten)
                    nc.vector.memset(h3d[32:64, :, :, 0:10:9], 0.0)
                    nc.vector.memset(h3d[0:32, :, 0:1, :], 0.0)
                    nc.vector.memset(h3d[64:96, :, 7:8, :], 0.0)
                    psp_2a = tc.tile_pool(name="ps_d2a", bufs=2, space="PSUM")
                    ps_2a = psp_2a.__enter__()
                    for bc in range(8):
                        bs = slice(bc * 32, bc * 32 + 32)
                        psa = ps_2a.tile([32, 32, 8, 8], f32, tag="psd2a")
                        for sb in range(4):
                            ss = slice(bc * 32 + sb * 8, bc * 32 + sb * 8 + 8)
                            nc.tensor.matmul(
                                psa[:, sb * 8 : sb * 8 + 8],
                                Wd2a[:],
                                h2[:, ss],
                                start=True,
                                stop=True,
                            )
                        nc.scalar.activation(
                            h3d[32:64, bs, :, 1:9], psa[:], AF.Identity, bias=bd2a[:]
                        )
                    nc.sync.dma_start(
                        out=h3d[0:32, :, 1:8, :], in_=h3d[32:64, :, 0:7, :]
                    )
                    nc.sync.dma_start(
                        out=h3d[64:96, :, 0:7, :], in_=h3d[32:64, :, 1:8, :]
                    )
                    psp_2a.__exit__(None, None, None)
                    psp_2b = tc.tile_pool(name="ps_d2b", bufs=2, space="PSUM")
                    ps_2b = psp_2b.__enter__()
                    for bc in range(8):
                        bs = slice(bc * 32, bc * 32 + 32)
                        s1 = p1w.tile([32, 16, 16, 32], bf16, tag="s1")
                        psb = ps_2b.tile([128, 32, 8, 8], f32, tag="psd2b")
                        for sb in range(4):
                            ss = slice(bc * 32 + sb * 8, bc * 32 + sb * 8 + 8)
                            for tx in range(3):
                                nc.tensor.matmul(
                                    psb[:, sb * 8 : sb * 8 + 8],
                                    Wd2b[:, tx, :],
                                    h3d[:, ss, :, tx : tx + 8],
                                    start=(tx == 0),
                                    stop=(tx == 2),
                                )
                        for ry in range(2):
                            for rx in range(2):
                                dst = s1[:, ry : 16 : 2, rx : 16 : 2, :]
                                nc.scalar.activation(
                                    dst.transpose([0, 3, 1, 2]),
                                    psb[(ry * 2 + rx) * 32 : (ry * 2 + rx) * 32 + 32],
                                    AF.Identity,
                                    bias=bd2b[:],
                                )
                        nc.sync.dma_start(out=mid1[bc], in_=s1[:])
                    psp_2b.__exit__(None, None, None)

            # ---------------- P2: switched deconvs ----------------
            # 64-sample quarters; g-halves merged in PSUM partitions so the
            # evac / mask / residual-add ops run on 64 partitions per instr.
            # Two quarters pair-packed per 128-partition tile: partition row
            # p = h*64 + g*32 + c (h = quarter-in-pair, g = 32-sample group).
            # Every elementwise op covers 128 samples; conv psums use all 4
            # PE column positions (0/32/64/96).
            with (
                tc.tile_pool(name="p2", bufs=1) as p2,
                tc.tile_pool(name="p2w", bufs=2) as p2w,
                tc.tile_pool(name="p2s", bufs=1) as p2s,
                tc.tile_pool(name="p2h", bufs=1) as p2h,
                tc.tile_pool(name="ps_sw1", bufs=4, space="PSUM") as ps_sw1,
                tc.tile_pool(name="ps_sw2", bufs=4, space="PSUM") as ps_sw2,
            ):
                for qp in range(2):  # pairs of 64-sample quarters
                    x4 = p2.tile([128, 16, 16, 32], bf16, tag="x4")
                    for hg in range(4):
                        nc.sync.dma_start(
                            out=x4[hg * 32 : hg * 32 + 32], in_=mid1[4 * qp + hg]
                        )
                    mzqt = p2.tile([128, 4, 32], bf16, tag="mzqt")
                    nc.sync.dma_start(out=mzqt[:], in_=mzQ_d[qp])
                    for s in range(4):
                        # hrelu and tt have disjoint lifetimes; share one
                        # 2-buffer tag so they alternate the same memory
                        hrelu = p2w.tile([128, 16, 16, 32], bf16, tag="sc")
                        tt = p2w.tile([128, 16, 16, 32], bf16, tag="sc")
                        h4s = [
                            p2h.tile([128, 16, 16, 32], bf16, tag=f"h4{hg}",
                                     name=f"h4{hg}")
                            for hg in range(4)
                        ]
                        tt4s = [
                            p2h.tile([96, 16, 16, 32], bf16, tag=f"tt4{hg}",
                                     name=f"tt4{hg}")
                            for hg in range(4)
                        ]
                        nc.scalar.activation(hrelu[:], x4[:], AF.Relu)
                        for hg in range(4):
                            h4 = h4s[hg]
                            g32 = slice(hg * 32, hg * 32 + 32)
                            eng = nc.gpsimd if hg % 2 == 0 else nc.sync
                            # halo zeros (32-aligned partition bases)
                            nc.gpsimd.memset(h4[0:32, 0:1, :, :], 0.0)
                            nc.gpsimd.memset(h4[64:96, 0:1, :, :], 0.0)
                            nc.gpsimd.memset(h4[64:96, :, 0:1, :], 0.0)
                            nc.gpsimd.memset(h4[96:128, :, 0:1, :], 0.0)
                            # taps: 0:32=(0,1)y 32:64=(1,1) 64:96=(0,0)xy 96:128=(1,0)x
                            eng.dma_start(out=h4[32:64], in_=hrelu[g32])
                            eng.dma_start(
                                out=h4[0:32, 1:16, :, :], in_=hrelu[g32, 0:15, :, :]
                            )
                            eng.dma_start(
                                out=h4[64:96, 1:16, 1:16, :],
                                in_=hrelu[g32, 0:15, 0:15, :],
                            )
                            eng.dma_start(
                                out=h4[96:128, :, 1:16, :], in_=hrelu[g32, :, 0:15, :]
                            )
                        for q in range(8):  # 4-sample psum chunks
                            s4 = slice(q * 4, q * 4 + 4)
                            for yb in range(2):
                                yr = slice(yb * 8, yb * 8 + 8)
                                pss = ps_sw1.tile([128, 8, 16, 4], f32, tag="pss1")
                                for hg in range(4):
                                    nc.tensor.matmul(
                                        pss[hg * 32 : hg * 32 + 32],
                                        Wsw1[:, s, :],
                                        h4s[hg][:, yr, :, s4],
                                        start=True,
                                        stop=True,
                                        tile_position=(0, hg * 32),
                                    )
                                nc.scalar.activation(
                                    tt[:, yr, :, s4],
                                    pss[:],
                                    AF.Relu,
                                    bias=bsw1[:, s : s + 1],
                                )
                        mzb = (
                            mzqt[:, s, :]
                            .unsqueeze(1)
                            .unsqueeze(1)
                            .broadcast_to([128, 16, 16, 32])
                        )
                        nc.vector.tensor_tensor(tt[:], tt[:], mzb, ALU.mult)
                        for hg in range(4):
                            tt4 = tt4s[hg]
                            g18 = slice(hg * 32, hg * 32 + 18)
                            eng = nc.gpsimd if hg % 2 == 1 else nc.sync
                            # rows: 0:18=(1,1) 18:36=(0,1)y 36:54=(0,0)xy 54:72=(1,0)x
                            nc.gpsimd.memset(tt4[0:64, 0:1, :, :], 0.0)
                            nc.gpsimd.memset(tt4[32:64, :, 0:1, :], 0.0)
                            nc.gpsimd.memset(tt4[64:96, :, 0:1, :], 0.0)
                            nc.sync.dma_start(
                                out=tt4[72:75], in_=mzP_d[s, 4 * qp + hg]
                            )
                            eng.dma_start(out=tt4[0:18], in_=tt[g18])
                            eng.dma_start(
                                out=tt4[18:36, 1:16, :, :], in_=tt[g18, 0:15, :, :]
                            )
                            eng.dma_start(
                                out=tt4[36:54, 1:16, 1:16, :],
                                in_=tt[g18, 0:15, 0:15, :],
                            )
                            eng.dma_start(
                                out=tt4[54:72, :, 1:16, :], in_=tt[g18, :, 0:15, :]
                            )
                        for q in range(8):
                            s4 = slice(q * 4, q * 4 + 4)
                            for yb in range(2):
                                yr = slice(yb * 8, yb * 8 + 8)
                                ps2 = ps_sw2.tile([128, 8, 16, 4], f32, tag="pss2")
                                for hg in range(4):
                                    nc.tensor.matmul(
                                        ps2[hg * 32 : hg * 32 + 32],
                                        Wsw2[:, s, :],
                                        tt4s[hg][0:75, yr, :, s4],
                                        start=True,
                                        stop=True,
                                        tile_position=(0, hg * 32),
                                    )
                                nc.vector.tensor_tensor(
                                    x4[:, yr, :, s4], x4[:, yr, :, s4], ps2[:], ALU.add
                                )
                    # store relu(x4) -> mid2 (x-halo baked)
                    hstp = p2s.tile([128, 16, 18, 32], bf16, tag="hst")
                    nc.gpsimd.memset(hstp[:, :, 0:1, :], 0.0)
                    nc.gpsimd.memset(hstp[:, :, 17:18, :], 0.0)
                    nc.scalar.activation(hstp[:, :, 1:17, :], x4[:], AF.Relu)
                    for hg in range(4):
                        nc.sync.dma_start(
                            out=mid2[4 * qp + hg], in_=hstp[hg * 32 : hg * 32 + 32]
                        )

            # ---------------- P3: d3 ----------------
            # psd3 partition = rx*64 + ry*32 + c; evacs write the xm-split
            # s5r [128, 18y(halo), 2 xmHalf, 9 xd, 32s] on 64 partitions each
            with (
                tc.tile_pool(name="p3", bufs=2) as p3,
                tc.tile_pool(name="ps_d3", bufs=2, space="PSUM") as ps_d3,
            ):
                for qt in range(8):  # 32-sample chunks
                    h5 = p3.tile([96, 16, 18, 32], bf16, tag="h5")
                    nc.sync.dma_start(out=h5[32:64], in_=mid2[qt])
                    nc.sync.dma_start(out=h5[0:32, 1:16, :, :], in_=mid2[qt, :, 0:15, :, :])
                    nc.sync.dma_start(out=h5[64:96, 0:15, :, :], in_=mid2[qt, :, 1:16, :, :])
                    nc.vector.memset(h5[0:32, 0:1, :, :], 0.0)
                    nc.vector.memset(h5[64:96, 15:16, :, :], 0.0)
                    s5r = p3.tile([128, 18, 2, 9, 32], bf16, tag="s5r")
                    nc.gpsimd.memset(s5r[:, 0:1, :, :, :], 0.0)
                    nc.gpsimd.memset(s5r[:, 17:18, :, :, :], 0.0)
                    nc.gpsimd.memset(s5r[0:64, :, 0, 8:9, :], 0.0)
                    nc.gpsimd.memset(s5r[64:128, :, 0, 0:1, :], 0.0)
                    nc.gpsimd.memset(s5r[:, :, 1, 8:9, :], 0.0)
                    for q3 in range(4):  # 8-sample psum chunks
                        psd3 = ps_d3.tile([128, 16, 16, 8], f32, tag="psd3")
                        bs8 = slice(q3 * 8, q3 * 8 + 8)
                        for yb in range(4):
                            ys = slice(yb * 4, yb * 4 + 4)
                            for tx in range(3):
                                nc.tensor.matmul(
                                    psd3[:, ys, :, :],
                                    Wd3[:, tx, :],
                                    h5[:, ys, tx : tx + 16, bs8],
                                    start=(tx == 0),
                                    stop=(tx == 2),
                                )
                        # X3 = 4k+2e+rx+1 -> (xmHalf, xd) per (rx, e)
                        nc.scalar.activation(
                            s5r[0:64, 1:17, 0, 0:8, bs8], psd3[0:64, :, 0:16:2, :],
                            AF.Relu, bias=bd3r[0:64],
                        )
                        nc.scalar.activation(
                            s5r[0:64, 1:17, 1, 0:8, bs8], psd3[0:64, :, 1:16:2, :],
                            AF.Relu, bias=bd3r[0:64],
                        )
                        nc.vector.tensor_scalar(
                            s5r[64:128, 1:17, 1, 0:8, bs8], psd3[64:128, :, 0:16:2, :],
                            bd3r[64:128], 0.0, ALU.add, ALU.max,
                        )
                        nc.vector.tensor_scalar(
                            s5r[64:128, 1:17, 0, 1:9, bs8], psd3[64:128, :, 1:16:2, :],
                            bd3r[64:128], 0.0, ALU.add, ALU.max,
                        )
                    nc.sync.dma_start(out=mid3[qt], in_=s5r[:])

            # ---------------- P4: d4 ----------------
            with (
                tc.tile_pool(name="p4", bufs=2) as p4,
                tc.tile_pool(name="ps_d4", bufs=2, space="PSUM") as ps_d4,
            ):
                for ck in range(8):  # 32-sample chunks
                    # h6a/h6b: (xm, c)-partition y-halo replicas of d3 out
                    h6a = p4.tile([128, 18, 9, 32], bf16, tag="h6a")
                    h6b = p4.tile([128, 18, 9, 32], bf16, tag="h6b")
                    for xm in range(4):
                        rx, half = 1 - xm % 2, xm // 2
                        nc.sync.dma_start(
                            out=h6a[xm * 32 : xm * 32 + 32],
                            in_=mid3[ck, rx * 64 : rx * 64 + 32, :, half, :, :],
                        )
                        nc.sync.dma_start(
                            out=h6b[xm * 32 : xm * 32 + 32],
                            in_=mid3[ck, rx * 64 + 32 : rx * 64 + 64, :, half, :, :],
                        )
                    outSB = p4.tile([96, 16, 8, 32], bf16, tag="osb")
                    # rounds r=dl: (tensor, y-offset) = (b,0),(a,1),(b,1),(a,2)
                    RND = ((h6b, 0), (h6a, 1), (h6b, 1), (h6a, 2))
                    for xbb in range(8):
                        psd4 = ps_d4.tile([96, 16, 32], f32, tag="psd4")
                        for r in range(4):
                            hx, dd = RND[r]
                            nc.tensor.matmul(
                                psd4[:],
                                Wd4A[:, r, :],
                                hx[:, dd : dd + 16, xbb, :],
                                start=(r == 0),
                                stop=False,
                            )
                            nc.tensor.matmul(
                                psd4[:],
                                Wd4B[:, r, :],
                                hx[0:64, dd : dd + 16, xbb + 1, :],
                                start=False,
                                stop=(r == 3),
                            )
                        nc.scalar.activation(
                            outSB[:, :, xbb, :], psd4[:], AF.Identity, bias=bd4r[:],
                        )
                    nc.sync.dma_start(out=outD[ck], in_=outSB[:])

    nc.compile()
    return nc


# --------------------------------------------------------------------------
# entry point
# --------------------------------------------------------------------------


def kernel(z2, ys_index, zs, fc_latent_w, fc_latent_b,
           fcsw_w1, fcsw_b1, fcsw_w2, fcsw_b2,
           dcsw_w1, dcsw_b1, dcsw_w2, dcsw_b2,
           w_d1, b_d1, w_d2a, b_d2a, w_d2b, b_d2b,
           w_d3, b_d3, w_d4, b_d4, _trace=False, _want_res=False):
    from concourse import bass_utils

    inp = dict(z2=z2, ys_index=ys_index, zs=zs, fc_latent_w=fc_latent_w,
               fc_latent_b=fc_latent_b, fcsw_w1=fcsw_w1, fcsw_b1=fcsw_b1,
               fcsw_w2=fcsw_w2, fcsw_b2=fcsw_b2, dcsw_w1=dcsw_w1,
               dcsw_b1=dcsw_b1, dcsw_w2=dcsw_w2, dcsw_b2=dcsw_b2,
               w_d1=w_d1, b_d1=b_d1, w_d2a=w_d2a, b_d2a=b_d2a, w_d2b=w_d2b,
               b_d2b=b_d2b, w_d3=w_d3, b_d3=b_d3, w_d4=w_d4, b_d4=b_d4)
    if "nc" not in _NC_CACHE:
        _NC_CACHE["nc"] = _build_nc()
    nc = _NC_CACHE["nc"]
    wshared = _pack_weights(inp)
    in_maps = []
    for c in range(NCORES):
        m = dict(wshared)
        m.update(_per_core_inputs(inp, c))
        in_maps.append(m)
    res = bass_utils.run_bass_kernel_spmd(nc, in_maps, list(range(NCORES)),
                                          trace=_trace)
    out = np.empty((B, 3, 64, 64), np.float32)
    # per-core outD: [8ck, 96, 16y, 8xd, 32s]; partition p = jp*24 + g8*3 + oc
    # with g8 = (u*2+ry)*2+rx; y_out = 4*y + 2*u + ry; x_out = 8*xd + 2*jp + rx
    ov = out.reshape(NCORES, 8, 32, 3, 16, 4, 8, 4, 2)  # [c,ck,s,oc,y,yr,xd,jp,rx]
    for c in range(NCORES):
        a = np.asarray(res.results[c]["out"], np.float32).reshape(
            8, 4, 4, 2, 3, 16, 8, 32
        )  # [ck, jp, yr, rx, oc, y, xd, s]
        ov[c] = a.transpose(0, 7, 4, 5, 2, 6, 1, 3)
    if _want_res:
        return out, res
    return out

